# revision 16
# baseline (speedup 1.0000x reference)
"""Trainium2 Bass kernel for nn_MetaMixer_6717328851330.

Computation (see reference):
    p = x @ W_in.T ; h, gate = split(p) ; gate = silu(gate)
    h = causal_grouped_conv1d(h) + b_conv ; h = h * gate       (residual)
    hn = layernorm_I(h) ; m = silu(hn @ W_fc.T + b_fc) @ W_cp.T + b_cp
    y = (m + residual) @ W_out.T

Strategy: pure data-parallel over the 8192 tokens (B*L), 1024 tokens per
core, no collectives. The causal-conv left halo (3 tokens) is shipped
pre-computed from the host (haloh); the intra-core segment halo comes from
the previous segment's h tiles resident in SBUF.

All matmuls run in bf16 (tolerance is 2e-2; measured bf16 error ~5e-3),
which halves HBM weight traffic AND halves the PE's per-matmul stationary
weight-load time vs fp32r (2-byte vs 4-byte loads). Each weight block is
DMA'd once and consumed by both 512-token segments back-to-back. Bias adds
and LN statistics run on the scalar/vector engines instead of K=1 matmuls,
which on HW cost a full 512-row pass each on the PE (the bottleneck
engine: it runs at ~95% occupancy).

On-core layout: activations live as [channel, token] tiles so every matmul
contracts along SBUF partitions with no transposes (host pre-transposes x
and all weights). PSUM fp32 accumulation throughout; LN stats in fp32.
"""
import sys

sys.path.insert(0, "/opt/trn_rl_repo")
import ml_dtypes
import numpy as np

NCORES = 8
B, L, H, I, G, CK = 2, 4096, 1024, 2048, 8, 4
T = (B * L) // NCORES          # tokens per core
S = 512                        # token segment (= psum bank free dim)
NSEG = T // S
HK = H // 128                  # 8  k-chunks over hidden
IK = I // 128                  # 16 k-chunks over intermediate
QC = NCORES // B               # seq chunks per batch
EPS = 1e-5

_CACHE = {}


def _build():
    import concourse.bacc as bacc
    import concourse.mybir as mybir
    import concourse.tile as tile
    from concourse.alu_op_type import AluOpType

    f32 = mybir.dt.float32
    f32r = mybir.dt.float32r
    bf16 = mybir.dt.bfloat16
    AF = mybir.ActivationFunctionType
    MUL, ADD, SUB = AluOpType.mult, AluOpType.add, AluOpType.subtract

    nc = bacc.Bacc(None, target_bir_lowering=False)

    xT = nc.dram_tensor("xT", [H, T + 3], bf16, kind="ExternalInput")
    win = nc.dram_tensor("win", [128, G * 4096], bf16, kind="ExternalInput")
    cw = nc.dram_tensor("cw", [128, G * 2048], bf16, kind="ExternalInput")
    fcw = nc.dram_tensor("fcw", [128, 4 * 4096], bf16, kind="ExternalInput")
    cpw = nc.dram_tensor("cpw", [128, 8 * 2048], bf16, kind="ExternalInput")
    outw = nc.dram_tensor("outw", [128, 4 * 4096], bf16, kind="ExternalInput")
    haloh = nc.dram_tensor("haloh", [128, IK * 4], bf16, kind="ExternalInput")
    cbcol = nc.dram_tensor("cbcol", [128, IK], f32, kind="ExternalInput")
    cpbcol = nc.dram_tensor("cpbcol", [128, IK], f32, kind="ExternalInput")
    fcbcol = nc.dram_tensor("fcbcol", [128, HK], f32, kind="ExternalInput")
    sfccol = nc.dram_tensor("sfccol", [128, HK], f32, kind="ExternalInput")
    onesf = nc.dram_tensor("onesf", [128, 128], f32r, kind="ExternalInput")
    yT = nc.dram_tensor("yT", [H, T], f32, kind="ExternalOutput")

    with nc.allow_low_precision(reason="bf16 matmul pipeline"), \
         tile.TileContext(nc) as tc, \
         tc.tile_pool(name="sb", bufs=1) as sb, \
         tc.tile_pool(name="ps", bufs=1, space="PSUM") as ps:

        def mm_ps():
            return ps.tile([128, S], f32, tag="mm", bufs=5, name="mmps")

        # ---- inputs + constants. Segment-0 x slices first: the first
        # in_proj chain only needs those 1MB, not the full 2.1MB of x.
        ones128 = sb.tile([1, 128], f32r, tag="ones128", name="ones128")
        nc.sync.dma_start(ones128[:], onesf[0:1, :])
        xk = [[None] * HK for _ in range(NSEG)]
        for s in range(NSEG):
            for k in range(HK):
                t = sb.tile([128, S + 3], bf16, tag=f"x{s}_{k}", name=f"x{s}_{k}")
                nc.scalar.dma_start(t[:], xT[k * 128:(k + 1) * 128,
                                             s * S:s * S + S + 3])
                xk[s][k] = t
        carryall = sb.tile([128, IK * 4], bf16, tag="carryall", name="carryall")
        nc.scalar.dma_start(carryall[:], haloh[:])
        ones_col = sb.tile([128, 1], f32r, tag="ones_col", name="ones_col")
        nc.scalar.dma_start(ones_col[:], onesf[:, 0:1])
        # p-state warm-up: ~dozen throwaway matmuls on the ones tile so the
        # PE clock is ramped when the first real chain's operands land
        warm = ps.tile([128, S], f32, tag="pba", bufs=3, name="warmps")
        for w in range(10):
            nc.tensor.matmul(warm[:, 0:128], ones128[:], ones128[:],
                             start=(w == 0), stop=(w == 9))
        cb_t = sb.tile([128, IK], f32, tag="cbt", name="cbt")
        nc.scalar.dma_start(cb_t[:], cbcol[:])
        cpb_t = sb.tile([128, IK], f32, tag="cpbt", name="cpbt")
        nc.scalar.dma_start(cpb_t[:], cpbcol[:])
        fcb_t = sb.tile([128, HK], f32, tag="fcbt", name="fcbt")
        nc.scalar.dma_start(fcb_t[:], fcbcol[:])
        sfc_t = sb.tile([128, HK], f32, tag="sfct", name="sfct")
        nc.scalar.dma_start(sfc_t[:], sfccol[:])

        res = [[None] * NSEG for _ in range(IK)]
        sumt = [None] * NSEG
        sqsum = [None] * NSEG

        # ---- Phase A: in_proj + causal grouped conv + silu-gate, per group
        for g in range(G):
            wg = sb.tile([128, 4096], bf16, tag="wbig", bufs=3, name="wbig")
            if g == 0:
                # fast start: 8 chunk-sized DMAs so the first matmul only
                # waits on 128KB, not the full 1MB block
                for k in range(HK):
                    nc.sync.dma_start(wg[:, k * 512:(k + 1) * 512],
                                      win[:, k * 512:(k + 1) * 512])
            else:
                # half-block DMAs: the chain's first matmuls start when the
                # first 512KB lands
                nc.sync.dma_start(wg[:, 0:2048], win[:, g * 4096:g * 4096 + 2048])
                nc.sync.dma_start(wg[:, 2048:4096],
                                  win[:, g * 4096 + 2048:(g + 1) * 4096])
            cwt = sb.tile([128, 2048], bf16, tag="cw2", bufs=2, name="cwt")
            nc.scalar.dma_start(cwt[:], cw[:, g * 2048:(g + 1) * 2048])

            hts = [[None] * NSEG, [None] * NSEG]
            for m in range(2):
                i2 = 2 * g + m
                for s in range(NSEG):
                    pm = mm_ps()
                    for k in range(HK):
                        nc.tensor.matmul(pm[:],
                                         wg[:, k * 512 + m * 128:k * 512 + m * 128 + 128],
                                         xk[s][k][:, 3:3 + S],
                                         start=(k == 0), stop=(k == HK - 1))
                    ht = sb.tile([128, S + 3], bf16, tag="hT", bufs=6, name="hT")
                    nc.vector.tensor_copy(ht[:, 3:S + 3], pm[:])
                    if s == 0:
                        nc.vector.tensor_copy(ht[:, 0:3],
                                              carryall[:, i2 * 4:i2 * 4 + 3])
                    else:
                        nc.vector.tensor_copy(ht[:, 0:3],
                                              hts[m][s - 1][:, S:S + 3])
                    hts[m][s] = ht

            gss = [[None] * NSEG, [None] * NSEG]
            for m in range(2):
                for s in range(NSEG):
                    pg = mm_ps()
                    for k in range(HK):
                        nc.tensor.matmul(pg[:],
                                         wg[:, k * 512 + 256 + m * 128:k * 512 + 384 + m * 128],
                                         xk[s][k][:, 3:3 + S],
                                         start=(k == 0), stop=(k == HK - 1))
                    gs = sb.tile([128, S], bf16, tag="gsc", bufs=6, name="gsc")
                    nc.scalar.activation(gs[:], pg[:], AF.Silu)
                    gss[m][s] = gs

            for m in range(2):
                i2 = 2 * g + m
                for s in range(NSEG):
                    pc = mm_ps()
                    first = True
                    for cc in range(2):
                        for k in range(CK):
                            c0 = cc * 1024 + k * 256 + m * 128
                            nc.tensor.matmul(pc[:], cwt[:, c0:c0 + 128],
                                             hts[cc][s][:, k:k + S],
                                             start=first,
                                             stop=(cc == 1 and k == CK - 1))
                            first = False
                    tb = sb.tile([128, S], bf16, tag="tb", bufs=4, name="tb")
                    nc.scalar.activation(tb[:], pc[:], AF.Identity,
                                         bias=cb_t[:, i2:i2 + 1])
                    rs = sb.tile([128, S], bf16, tag=f"res{i2}_{s}",
                                 name=f"res{i2}_{s}")
                    nc.vector.tensor_tensor(rs[:], tb[:], gss[m][s][:], op=MUL)
                    res[i2][s] = rs

            # ---- LN statistics, part 1: running channel sums / sum-squares
            # interleaved with phase A (cross-partition reduction finishes
            # with a single ones-matmul each below; doing all 16 k-chunks as
            # ones-matmuls would cost 60 more PE passes on the bottleneck)
            for s in range(NSEG):
                r0, r1 = res[2 * g][s], res[2 * g + 1][s]
                q0 = sb.tile([128, S], f32r, tag="sq", bufs=4, name="sq")
                nc.scalar.activation(q0[:], r0[:], AF.Square)
                q1 = sb.tile([128, S], f32r, tag="sq", bufs=4, name="sq")
                nc.scalar.activation(q1[:], r1[:], AF.Square)
                if g == 0:
                    a = sb.tile([128, S], f32r, tag=f"accs{s}", bufs=2,
                                name="accs")
                    nc.vector.tensor_tensor(a[:], r0[:], r1[:], op=ADD)
                    sumt[s] = a
                    aq = sb.tile([128, S], f32r, tag=f"accq{s}", bufs=2,
                                 name="accq")
                    nc.vector.tensor_tensor(aq[:], q0[:], q1[:], op=ADD)
                    sqsum[s] = aq
                else:
                    a1 = sb.tile([128, S], f32r, tag=f"accs{s}", bufs=2,
                                 name="accs")
                    nc.vector.tensor_tensor(a1[:], sumt[s][:], r0[:], op=ADD)
                    a2 = sb.tile([128, S], f32r, tag=f"accs{s}", bufs=2,
                                 name="accs")
                    nc.vector.tensor_tensor(a2[:], a1[:], r1[:], op=ADD)
                    sumt[s] = a2
                    aq1 = sb.tile([128, S], f32r, tag=f"accq{s}", bufs=2,
                                  name="accq")
                    nc.vector.tensor_tensor(aq1[:], sqsum[s][:], q0[:], op=ADD)
                    aq2 = sb.tile([128, S], f32r, tag=f"accq{s}", bufs=2,
                                  name="accq")
                    nc.vector.tensor_tensor(aq2[:], aq1[:], q1[:], op=ADD)
                    sqsum[s] = aq2

        # ---- MLP c_fc, chains only for the first H-block: keeps the PE busy
        # while the LN row math (below) runs on vector/scalar
        def fc_chain(wb, j2, s):
            pm = mm_ps()
            for kk in range(IK):
                nc.tensor.matmul(pm[:],
                                 wb[:, kk * 256 + j2 * 128:kk * 256 + j2 * 128 + 128],
                                 res[kk][s][:],
                                 start=(kk == 0), stop=(kk == IK - 1))
            return pm

        def wblock(src, jp):
            wb = sb.tile([128, 4096], bf16, tag="wbig", bufs=3, name="wbig2")
            nc.sync.dma_start(wb[:, 0:2048], src[:, jp * 4096:jp * 4096 + 2048])
            nc.sync.dma_start(wb[:, 2048:4096],
                              src[:, jp * 4096 + 2048:(jp + 1) * 4096])
            return wb

        fcwb0 = wblock(fcw, 0)
        jp0_psums = {}
        jp0_psums[(0, 0)] = fc_chain(fcwb0, 0, 0)

        # ---- LN statistics, part 2: finish stats. The PE reaches the stat
        # matmuls ~8us into the jp0 chains above, by which time the running
        # accumulators have drained; the j2=1 chains below then cover the
        # serial row-math latency (incl. the Sqrt act-table load) before the
        # pba broadcasts need arow/brow.
        bcA = [None] * NSEG
        bcB = [None] * NSEG
        stat_ps = {}

        def emit_stats(s):
            pst0 = ps.tile([1, S], f32, tag="pba", bufs=3, name="statps")
            nc.tensor.matmul(pst0[:], ones_col[:], sumt[s][:],
                             start=True, stop=True)
            pst1 = ps.tile([1, S], f32, tag="pba", bufs=3, name="statps")
            nc.tensor.matmul(pst1[:], ones_col[:], sqsum[s][:],
                             start=True, stop=True)
            stat_ps[s] = (pst0, pst1)

        emit_stats(0)
        jp0_psums[(0, 1)] = fc_chain(fcwb0, 0, 1)
        emit_stats(1)
        rows = {}
        for s in range(NSEG):
            pst0, pst1 = stat_ps[s]
            mneg = sb.tile([1, S], f32r, tag="lnrow", bufs=6, name="mneg")
            nc.vector.tensor_scalar(mneg[:], pst0[:], -1.0 / I, None, op0=MUL)
            msq = sb.tile([1, S], f32, tag="lnrow", bufs=6, name="msq")
            nc.scalar.activation(msq[:], pst0[:], AF.Square)
            nc.vector.tensor_scalar(msq[:], msq[:], 1.0 / I, None, op0=MUL)
            vrow = sb.tile([1, S], f32, tag="lnrow", bufs=6, name="vrow")
            nc.vector.tensor_tensor(vrow[:], pst1[:], msq[:], op=SUB)
            nc.vector.tensor_scalar(vrow[:], vrow[:], 1.0 / I, EPS,
                                    op0=MUL, op1=ADD)
            sd = sb.tile([1, S], f32, tag="lnrow", bufs=6, name="sd")
            nc.scalar.activation(sd[:], vrow[:], AF.Sqrt)
            arow = sb.tile([1, S], f32r, tag="lnrow", bufs=6, name="arow")
            nc.vector.reciprocal(arow[:], sd[:])                    # rstd
            brow = sb.tile([1, S], f32r, tag="lnrow", bufs=6, name="brow")
            nc.vector.tensor_tensor(brow[:], mneg[:], arow[:], op=MUL)
            rows[s] = (arow, brow)
        for s in range(NSEG):
            jp0_psums[(1, s)] = fc_chain(fcwb0, 1, s)
        for s in range(NSEG):
            arow, brow = rows[s]
            pa = ps.tile([128, S], f32, tag="pba", bufs=3, name="pbaps")
            nc.tensor.matmul(pa[:], ones128[:], arow[:], start=True, stop=True)
            bA = sb.tile([128, S], f32, tag=f"bcA{s}", name=f"bcA{s}")
            nc.vector.tensor_copy(bA[:], pa[:])
            bcA[s] = bA
            pb = ps.tile([128, S], f32, tag="pba", bufs=3, name="pbaps")
            nc.tensor.matmul(pb[:], ones128[:], brow[:], start=True, stop=True)
            bB = sb.tile([128, S], f32, tag=f"bcB{s}", name=f"bcB{s}")
            nc.vector.tensor_copy(bB[:], pb[:])
            bcB[s] = bB

        # ---- MLP c_fc: hn = (psum - mean*S_fc_row)*rstd, done per tile as
        # psum*bcA + S_fc[j]*bcB, then silu(+folded bias)
        m1 = [[None] * NSEG for _ in range(HK)]

        def fc_finish(pm, j, s):
            t1 = sb.tile([128, S], bf16, tag="fct1", bufs=2, name="fct1")
            nc.vector.tensor_scalar(t1[:], bcB[s][:], sfc_t[:, j:j + 1], None,
                                    op0=MUL)
            t2 = sb.tile([128, S], bf16, tag="fct2", bufs=2, name="fct2")
            nc.vector.tensor_tensor(t2[:], pm[:], bcA[s][:], op=MUL)
            t3 = sb.tile([128, S], bf16, tag="fct3", bufs=2, name="fct3")
            nc.vector.tensor_tensor(t3[:], t2[:], t1[:], op=ADD)
            o = sb.tile([128, S], bf16, tag=f"m1_{j}_{s}", name=f"m1_{j}_{s}")
            nc.scalar.activation(o[:], t3[:], AF.Silu, bias=fcb_t[:, j:j + 1])
            m1[j][s] = o

        for j2 in range(2):
            for s in range(NSEG):
                fc_finish(jp0_psums[(j2, s)], j2, s)
        for jp in range(1, 4):
            wb = wblock(fcw, jp)
            for j2 in range(2):
                for s in range(NSEG):
                    pm = fc_chain(wb, j2, s)
                    fc_finish(pm, jp * 2 + j2, s)

        # ---- MLP c_proj (H -> I) + bias + residual add
        oin = [[None] * NSEG for _ in range(IK)]
        for ip in range(8):
            wb = sb.tile([128, 2048], bf16, tag="cw2", bufs=2, name="cpwb")
            nc.scalar.dma_start(wb[:], cpw[:, ip * 2048:(ip + 1) * 2048])
            for i2 in range(2):
                i = ip * 2 + i2
                for s in range(NSEG):
                    pm = mm_ps()
                    for kk in range(HK):
                        nc.tensor.matmul(pm[:],
                                         wb[:, kk * 256 + i2 * 128:kk * 256 + i2 * 128 + 128],
                                         m1[kk][s][:],
                                         start=(kk == 0), stop=(kk == HK - 1))
                    tb2 = sb.tile([128, S], bf16, tag="tb", bufs=4, name="tb2")
                    nc.scalar.activation(tb2[:], pm[:], AF.Identity,
                                         bias=cpb_t[:, i:i + 1])
                    oi = sb.tile([128, S], bf16, tag=f"oin{i}_{s}",
                                 name=f"oin{i}_{s}")
                    nc.vector.tensor_tensor(oi[:], tb2[:], res[i][s][:], op=ADD)
                    oin[i][s] = oi

        # ---- out_proj (I -> H)
        for jp in range(4):
            wb = wblock(outw, jp)
            for j2 in range(2):
                j = jp * 2 + j2
                for s in range(NSEG):
                    pm = mm_ps()
                    for kk in range(IK):
                        nc.tensor.matmul(pm[:],
                                         wb[:, kk * 256 + j2 * 128:kk * 256 + j2 * 128 + 128],
                                         oin[kk][s][:],
                                         start=(kk == 0), stop=(kk == IK - 1))
                    yo = sb.tile([128, S], f32, tag="yo", bufs=3, name="yo")
                    nc.vector.tensor_copy(yo[:], pm[:])
                    nc.scalar.dma_start(yT[j * 128:(j + 1) * 128, s * S:(s + 1) * S],
                                        yo[:])

    nc.compile()
    return nc


def _pack(inputs):
    bf = ml_dtypes.bfloat16
    f = lambda name: np.asarray(inputs[name], np.float32)
    hs = np.ascontiguousarray(f("hidden_states"))
    wT = np.ascontiguousarray(f("in_proj_w").T)                 # [H, 2I]
    winp = np.empty((H, 2 * I), np.float32)
    for g in range(G):
        winp[:, g * 512:g * 512 + 256] = wT[:, g * 256:(g + 1) * 256]
        winp[:, g * 512 + 256:(g + 1) * 512] = wT[:, I + g * 256:I + (g + 1) * 256]
    # block layouts: [128, nblocks*4096]; block g holds 8 lhsT chunk groups
    # [128, 512] = [h_m0, h_m1, gate_m0, gate_m1] so each phase-block is one
    # contiguous DMA
    winb = np.ascontiguousarray(
        winp.reshape(HK, 128, G, 512).transpose(1, 2, 0, 3)
        .reshape(128, G * 4096).astype(bf))
    # layernorm gamma/beta folded into c_fc (exact): silu((hn*g+b) @ W.T + c)
    # = silu(hn @ (W*g).T + (c + W @ b))
    fcw_eff = f("fc_w") * f("ln_g")[None, :]
    fcb_eff = f("fc_b") + f("fc_w") @ f("ln_b")
    sfc_col = np.ascontiguousarray(
        fcw_eff.sum(axis=1, dtype=np.float64).astype(np.float32)
        .reshape(HK, 128).T)
    # fc/out blocks keyed by jp (pair of 128-col output blocks): block jp =
    # [128, 16 kk * 256], chunk (kk, j2) at col kk*256 + j2*128
    fcwb = np.ascontiguousarray(
        fcw_eff.T.reshape(IK, 128, 4, 2, 128).transpose(1, 2, 0, 3, 4)
        .reshape(128, 4 * 4096).astype(bf))
    outwb = np.ascontiguousarray(
        f("out_w").T.reshape(IK, 128, 4, 2, 128).transpose(1, 2, 0, 3, 4)
        .reshape(128, 4 * 4096).astype(bf))
    # cproj blocks keyed by ip: block ip = [128, 8 kk * 256]
    cpwb = np.ascontiguousarray(
        f("cproj_w").T.reshape(HK, 128, 8, 2, 128).transpose(1, 2, 0, 3, 4)
        .reshape(128, 8 * 2048).astype(bf))
    v = f("conv_w").reshape(G, 256, 2, 128, CK)                 # [g, j, cc, i, k]
    cwp = np.ascontiguousarray(
        v.transpose(3, 0, 2, 4, 1).reshape(128, G * 2048).astype(bf))
    shared = dict(
        win=winb, cw=cwp, fcw=fcwb, cpw=cpwb, outw=outwb,
        cbcol=np.ascontiguousarray(f("conv_b").reshape(IK, 128).T),
        cpbcol=np.ascontiguousarray(f("cproj_b").reshape(IK, 128).T),
        fcbcol=np.ascontiguousarray(fcb_eff.reshape(HK, 128).T),
        sfccol=sfc_col,
        onesf=np.ones((128, 128), np.float32),
    )
    ipw_h = f("in_proj_w")[:I]                                  # [I, H]
    in_maps = []
    for c in range(NCORES):
        b, q = divmod(c, QC)
        own = hs[b, q * T:(q + 1) * T]                          # [T, H]
        prev = (np.zeros((3, H), np.float32) if q == 0
                else hs[b, q * T - 3:q * T])
        xTc = np.ascontiguousarray(
            np.concatenate([prev, own], 0).T.astype(bf))        # [H, T+3]
        hh = np.zeros((IK, 128, 4), np.float32)
        hh[:, :, 0:3] = (ipw_h @ prev.T).reshape(IK, 128, 3)    # halo h columns
        hh = np.ascontiguousarray(
            hh.transpose(1, 0, 2).reshape(128, IK * 4).astype(bf))
        in_maps.append(dict(xT=xTc, haloh=hh, **shared))
    return in_maps


def _run(inputs, trace=False):
    from concourse.bass_utils import run_bass_kernel_spmd

    nc = _CACHE.get("nc")
    if nc is None:
        nc = _build()
        _CACHE["nc"] = nc
    in_maps = _pack(inputs)
    try:
        res = run_bass_kernel_spmd(nc, in_maps, core_ids=list(range(NCORES)),
                                   trace=trace)
    except Exception:
        # transient NRT_EXEC_UNIT_UNRECOVERABLE has been observed once after a
        # wedged prior run; one retry has always succeeded
        res = run_bass_kernel_spmd(nc, in_maps, core_ids=list(range(NCORES)),
                                   trace=trace)
    y = np.empty((B, L, H), np.float32)
    for c in range(NCORES):
        b, q = divmod(c, QC)
        y[b, q * T:(q + 1) * T, :] = res.results[c]["yT"].T
    return y, res


def kernel(**inputs) -> np.ndarray:
    y, _ = _run(inputs, trace=False)
    return y


# revision 23
# speedup vs baseline: 1.0174x; 1.0174x over previous
"""Trainium2 Bass kernel for nn_MetaMixer_6717328851330.

Computation (see reference):
    p = x @ W_in.T ; h, gate = split(p) ; gate = silu(gate)
    h = causal_grouped_conv1d(h) + b_conv ; h = h * gate       (residual)
    hn = layernorm_I(h) ; m = silu(hn @ W_fc.T + b_fc) @ W_cp.T + b_cp
    y = (m + residual) @ W_out.T

Strategy: pure data-parallel over the 8192 tokens (B*L), 1024 tokens per
core, no collectives. The causal-conv left halo (3 tokens) is shipped
pre-computed from the host (haloh); the intra-core segment halo comes from
the previous segment's h tiles resident in SBUF.

All matmuls run in bf16 (tolerance is 2e-2; measured bf16 error ~5e-3),
which halves HBM weight traffic AND halves the PE's per-matmul stationary
weight-load time vs fp32r (2-byte vs 4-byte loads). Each weight block is
DMA'd once and consumed by both 512-token segments back-to-back. Bias adds
and LN statistics run on the scalar/vector engines instead of K=1 matmuls,
which on HW cost a full 512-row pass each on the PE (the bottleneck
engine: it runs at ~95% occupancy).

On-core layout: activations live as [channel, token] tiles so every matmul
contracts along SBUF partitions with no transposes (host pre-transposes x
and all weights). PSUM fp32 accumulation throughout; LN stats in fp32.
"""
import sys

sys.path.insert(0, "/opt/trn_rl_repo")
import ml_dtypes
import numpy as np

NCORES = 8
B, L, H, I, G, CK = 2, 4096, 1024, 2048, 8, 4
T = (B * L) // NCORES          # tokens per core
S = 512                        # token segment (= psum bank free dim)
NSEG = T // S
HK = H // 128                  # 8  k-chunks over hidden
IK = I // 128                  # 16 k-chunks over intermediate
QC = NCORES // B               # seq chunks per batch
EPS = 1e-5

_CACHE = {}


def _build():
    import concourse.bacc as bacc
    import concourse.mybir as mybir
    import concourse.tile as tile
    from concourse.alu_op_type import AluOpType

    f32 = mybir.dt.float32
    f32r = mybir.dt.float32r
    bf16 = mybir.dt.bfloat16
    AF = mybir.ActivationFunctionType
    MUL, ADD, SUB = AluOpType.mult, AluOpType.add, AluOpType.subtract

    nc = bacc.Bacc(None, target_bir_lowering=False)

    xT = nc.dram_tensor("xT", [H, T + 3], bf16, kind="ExternalInput")
    win = nc.dram_tensor("win", [128, G * 4096], bf16, kind="ExternalInput")
    cw = nc.dram_tensor("cw", [128, G * 2048], bf16, kind="ExternalInput")
    fcw = nc.dram_tensor("fcw", [128, 4 * 4096], bf16, kind="ExternalInput")
    cpw = nc.dram_tensor("cpw", [128, 8 * 2048], bf16, kind="ExternalInput")
    outw = nc.dram_tensor("outw", [128, 4 * 4096], bf16, kind="ExternalInput")
    haloh = nc.dram_tensor("haloh", [128, IK * 4], bf16, kind="ExternalInput")
    cbcol = nc.dram_tensor("cbcol", [128, IK], f32, kind="ExternalInput")
    cpbcol = nc.dram_tensor("cpbcol", [128, IK], f32, kind="ExternalInput")
    fcbcol = nc.dram_tensor("fcbcol", [128, HK], f32, kind="ExternalInput")
    sfccol = nc.dram_tensor("sfccol", [128, HK], f32, kind="ExternalInput")
    onesf = nc.dram_tensor("onesf", [128, 128], f32r, kind="ExternalInput")
    yT = nc.dram_tensor("yT", [H, T], f32, kind="ExternalOutput")

    with nc.allow_low_precision(reason="bf16 matmul pipeline"), \
         tile.TileContext(nc) as tc, \
         tc.tile_pool(name="sb", bufs=1) as sb, \
         tc.tile_pool(name="ps", bufs=1, space="PSUM") as ps:

        def mm_ps():
            return ps.tile([128, S], f32, tag="mm", bufs=5, name="mmps")

        # ---- inputs + constants. Segment-0 x slices first: the first
        # in_proj chain only needs those 1MB, not the full 2.1MB of x.
        ones128 = sb.tile([1, 128], f32r, tag="ones128", name="ones128")
        nc.sync.dma_start(ones128[:], onesf[0:1, :])
        xk = [[None] * HK for _ in range(NSEG)]
        for s in range(NSEG):
            for k in range(HK):
                t = sb.tile([128, S + 3], bf16, tag=f"x{s}_{k}", name=f"x{s}_{k}")
                nc.scalar.dma_start(t[:], xT[k * 128:(k + 1) * 128,
                                             s * S:s * S + S + 3])
                xk[s][k] = t
        carryall = sb.tile([128, IK * 4], bf16, tag="carryall", name="carryall")
        nc.scalar.dma_start(carryall[:], haloh[:])
        ones_col = sb.tile([128, 1], f32r, tag="ones_col", name="ones_col")
        nc.scalar.dma_start(ones_col[:], onesf[:, 0:1])
        # p-state warm-up: ~dozen throwaway matmuls on the ones tile so the
        # PE clock is ramped when the first real chain's operands land
        warm = ps.tile([128, S], f32, tag="pba", bufs=3, name="warmps")
        for w in range(10):
            nc.tensor.matmul(warm[:, 0:128], ones128[:], ones128[:],
                             start=(w == 0), stop=(w == 9))
        cb_t = sb.tile([128, IK], f32, tag="cbt", name="cbt")
        nc.scalar.dma_start(cb_t[:], cbcol[:])
        cpb_t = sb.tile([128, IK], f32, tag="cpbt", name="cpbt")
        nc.scalar.dma_start(cpb_t[:], cpbcol[:])
        fcb_t = sb.tile([128, HK], f32, tag="fcbt", name="fcbt")
        nc.scalar.dma_start(fcb_t[:], fcbcol[:])
        sfc_t = sb.tile([128, HK], f32, tag="sfct", name="sfct")
        nc.scalar.dma_start(sfc_t[:], sfccol[:])

        res = [[None] * NSEG for _ in range(IK)]
        sumt = [None] * NSEG
        sqsum = [None] * NSEG

        # ---- Phase A: in_proj + causal grouped conv + silu-gate, per group
        for g in range(G):
            wg = sb.tile([128, 4096], bf16, tag="wbig", bufs=3, name="wbig")
            if g == 0:
                # fast start: 8 chunk-sized DMAs so the first matmul only
                # waits on 128KB, not the full 1MB block
                for k in range(HK):
                    nc.sync.dma_start(wg[:, k * 512:(k + 1) * 512],
                                      win[:, k * 512:(k + 1) * 512])
            else:
                # half-block DMAs: the chain's first matmuls start when the
                # first 512KB lands
                nc.sync.dma_start(wg[:, 0:2048], win[:, g * 4096:g * 4096 + 2048])
                nc.sync.dma_start(wg[:, 2048:4096],
                                  win[:, g * 4096 + 2048:(g + 1) * 4096])
            cwt = sb.tile([128, 2048], bf16, tag="cw2", bufs=2, name="cwt")
            nc.scalar.dma_start(cwt[:], cw[:, g * 2048:(g + 1) * 2048])

            hts = [[None] * NSEG, [None] * NSEG]
            for m in range(2):
                i2 = 2 * g + m
                for s in range(NSEG):
                    pm = mm_ps()
                    for k in range(HK):
                        nc.tensor.matmul(pm[:],
                                         wg[:, k * 512 + m * 128:k * 512 + m * 128 + 128],
                                         xk[s][k][:, 3:3 + S],
                                         start=(k == 0), stop=(k == HK - 1))
                    ht = sb.tile([128, S + 3], bf16, tag="hT", bufs=5, name="hT")
                    nc.vector.tensor_copy(ht[:, 3:S + 3], pm[:])
                    if s == 0:
                        nc.vector.tensor_copy(ht[:, 0:3],
                                              carryall[:, i2 * 4:i2 * 4 + 3])
                    else:
                        nc.vector.tensor_copy(ht[:, 0:3],
                                              hts[m][s - 1][:, S:S + 3])
                    hts[m][s] = ht

            gss = [[None] * NSEG, [None] * NSEG]
            for m in range(2):
                for s in range(NSEG):
                    pg = mm_ps()
                    for k in range(HK):
                        nc.tensor.matmul(pg[:],
                                         wg[:, k * 512 + 256 + m * 128:k * 512 + 384 + m * 128],
                                         xk[s][k][:, 3:3 + S],
                                         start=(k == 0), stop=(k == HK - 1))
                    gs = sb.tile([128, S], bf16, tag="gsc", bufs=4, name="gsc")
                    nc.scalar.activation(gs[:], pg[:], AF.Silu)
                    gss[m][s] = gs

            for m in range(2):
                i2 = 2 * g + m
                for s in range(NSEG):
                    pc = mm_ps()
                    first = True
                    for cc in range(2):
                        for k in range(CK):
                            c0 = cc * 1024 + k * 256 + m * 128
                            nc.tensor.matmul(pc[:], cwt[:, c0:c0 + 128],
                                             hts[cc][s][:, k:k + S],
                                             start=first,
                                             stop=(cc == 1 and k == CK - 1))
                            first = False
                    tb = sb.tile([128, S], bf16, tag="tb", bufs=3, name="tb")
                    nc.scalar.activation(tb[:], pc[:], AF.Identity,
                                         bias=cb_t[:, i2:i2 + 1])
                    rs = sb.tile([128, S], bf16, tag=f"res{i2}_{s}",
                                 name=f"res{i2}_{s}")
                    nc.vector.tensor_tensor(rs[:], tb[:], gss[m][s][:], op=MUL)
                    res[i2][s] = rs

            # ---- LN statistics, part 1: running channel sums / sum-squares
            # interleaved with phase A (cross-partition reduction finishes
            # with a single ones-matmul each below; doing all 16 k-chunks as
            # ones-matmuls would cost 60 more PE passes on the bottleneck)
            for s in range(NSEG):
                r0, r1 = res[2 * g][s], res[2 * g + 1][s]
                q0 = sb.tile([128, S], f32r, tag="sq", bufs=3, name="sq")
                nc.scalar.activation(q0[:], r0[:], AF.Square)
                q1 = sb.tile([128, S], f32r, tag="sq", bufs=3, name="sq")
                nc.scalar.activation(q1[:], r1[:], AF.Square)
                if g == 0:
                    a = sb.tile([128, S], f32r, tag=f"accs{s}", bufs=2,
                                name="accs")
                    nc.vector.tensor_tensor(a[:], r0[:], r1[:], op=ADD)
                    sumt[s] = a
                    aq = sb.tile([128, S], f32r, tag=f"accq{s}", bufs=2,
                                 name="accq")
                    nc.vector.tensor_tensor(aq[:], q0[:], q1[:], op=ADD)
                    sqsum[s] = aq
                else:
                    a1 = sb.tile([128, S], f32r, tag=f"accs{s}", bufs=2,
                                 name="accs")
                    nc.vector.tensor_tensor(a1[:], sumt[s][:], r0[:], op=ADD)
                    a2 = sb.tile([128, S], f32r, tag=f"accs{s}", bufs=2,
                                 name="accs")
                    nc.vector.tensor_tensor(a2[:], a1[:], r1[:], op=ADD)
                    sumt[s] = a2
                    aq1 = sb.tile([128, S], f32r, tag=f"accq{s}", bufs=2,
                                  name="accq")
                    nc.vector.tensor_tensor(aq1[:], sqsum[s][:], q0[:], op=ADD)
                    aq2 = sb.tile([128, S], f32r, tag=f"accq{s}", bufs=2,
                                  name="accq")
                    nc.vector.tensor_tensor(aq2[:], aq1[:], q1[:], op=ADD)
                    sqsum[s] = aq2

        # ---- MLP c_fc, chains only for the first H-block: keeps the PE busy
        # while the LN row math (below) runs on vector/scalar
        def fc_chain(wb, j2, s):
            pm = mm_ps()
            for kk in range(IK):
                nc.tensor.matmul(pm[:],
                                 wb[:, kk * 256 + j2 * 128:kk * 256 + j2 * 128 + 128],
                                 res[kk][s][:],
                                 start=(kk == 0), stop=(kk == IK - 1))
            return pm

        def wblock(src, jp):
            wb = sb.tile([128, 4096], bf16, tag="wbig", bufs=3, name="wbig2")
            nc.sync.dma_start(wb[:, 0:2048], src[:, jp * 4096:jp * 4096 + 2048])
            nc.sync.dma_start(wb[:, 2048:4096],
                              src[:, jp * 4096 + 2048:(jp + 1) * 4096])
            return wb

        fcwb0 = wblock(fcw, 0)
        jp0_psums = {}
        jp0_psums[(0, 0)] = fc_chain(fcwb0, 0, 0)

        # ---- LN statistics, part 2: finish stats. The PE reaches the stat
        # matmuls ~8us into the jp0 chains above, by which time the running
        # accumulators have drained; the j2=1 chains below then cover the
        # serial row-math latency (incl. the Sqrt act-table load) before the
        # pba broadcasts need arow/brow.
        bcA = [None] * NSEG
        bcB = [None] * NSEG
        stat_ps = {}

        def emit_stats(s):
            pst0 = ps.tile([1, S], f32, tag="pba", bufs=3, name="statps")
            nc.tensor.matmul(pst0[:], ones_col[:], sumt[s][:],
                             start=True, stop=True)
            pst1 = ps.tile([1, S], f32, tag="pba", bufs=3, name="statps")
            nc.tensor.matmul(pst1[:], ones_col[:], sqsum[s][:],
                             start=True, stop=True)
            stat_ps[s] = (pst0, pst1)

        emit_stats(0)
        jp0_psums[(0, 1)] = fc_chain(fcwb0, 0, 1)
        emit_stats(1)
        # row math level-by-level across both segments: the two Sqrts stay
        # adjacent on the scalar engine (one act-table load, not two) and the
        # reciprocal uses the fast DVE approximation (exact reciprocal() is
        # 3.3us per row and sat on the critical path)
        rows = {}
        mnegs, msqs, vrows, sds = {}, {}, {}, {}
        for s in range(NSEG):
            mneg = sb.tile([1, S], f32r, tag="mneg", bufs=2, name="mneg")
            nc.vector.tensor_scalar(mneg[:], stat_ps[s][0][:], -1.0 / I,
                                    None, op0=MUL)
            mnegs[s] = mneg
        for s in range(NSEG):
            msq = sb.tile([1, S], f32, tag="msq", bufs=2, name="msq")
            nc.scalar.activation(msq[:], stat_ps[s][0][:], AF.Square)
            msqs[s] = msq
        for s in range(NSEG):
            msq = msqs[s]
            nc.vector.tensor_scalar(msq[:], msq[:], 1.0 / I, None, op0=MUL)
            vrow = sb.tile([1, S], f32, tag="vrow", bufs=2, name="vrow")
            nc.vector.tensor_tensor(vrow[:], stat_ps[s][1][:], msq[:], op=SUB)
            nc.vector.tensor_scalar(vrow[:], vrow[:], 1.0 / I, EPS,
                                    op0=MUL, op1=ADD)
            vrows[s] = vrow
        for s in range(NSEG):
            sd = sb.tile([1, S], f32, tag="sd", bufs=2, name="sd")
            nc.scalar.activation(sd[:], vrows[s][:], AF.Sqrt)
            sds[s] = sd
        for s in range(NSEG):
            ar32 = sb.tile([1, S], f32, tag="ar32", bufs=2, name="ar32")
            nc.vector.reciprocal_approx_fast(ar32[:], sds[s][:])    # rstd
            arow = sb.tile([1, S], f32r, tag="arow", bufs=2, name="arow")
            nc.vector.tensor_copy(arow[:], ar32[:])
            brow = sb.tile([1, S], f32r, tag="brow", bufs=2, name="brow")
            nc.vector.tensor_tensor(brow[:], mnegs[s][:], arow[:], op=MUL)
            rows[s] = (arow, brow)
        for s in range(NSEG):
            jp0_psums[(1, s)] = fc_chain(fcwb0, 1, s)
        for s in range(NSEG):
            arow, brow = rows[s]
            pa = ps.tile([128, S], f32, tag="pba", bufs=3, name="pbaps")
            nc.tensor.matmul(pa[:], ones128[:], arow[:], start=True, stop=True)
            bA = sb.tile([128, S], bf16, tag=f"bcA{s}", name=f"bcA{s}")
            nc.vector.tensor_copy(bA[:], pa[:])
            bcA[s] = bA
            pb = ps.tile([128, S], f32, tag="pba", bufs=3, name="pbaps")
            nc.tensor.matmul(pb[:], ones128[:], brow[:], start=True, stop=True)
            bB = sb.tile([128, S], bf16, tag=f"bcB{s}", name=f"bcB{s}")
            nc.vector.tensor_copy(bB[:], pb[:])
            bcB[s] = bB

        # ---- MLP c_fc: hn = (psum - mean*S_fc_row)*rstd, done per tile as
        # psum*bcA + S_fc[j]*bcB, then silu(+folded bias)
        m1 = [[None] * NSEG for _ in range(HK)]

        def fc_finish(pm, j, s):
            t1 = sb.tile([128, S], bf16, tag="fct1", bufs=2, name="fct1")
            nc.vector.tensor_scalar(t1[:], bcB[s][:], sfc_t[:, j:j + 1], None,
                                    op0=MUL)
            t2 = sb.tile([128, S], bf16, tag="fct2", bufs=2, name="fct2")
            nc.vector.tensor_tensor(t2[:], pm[:], bcA[s][:], op=MUL)
            t3 = sb.tile([128, S], bf16, tag="fct3", bufs=2, name="fct3")
            nc.vector.tensor_tensor(t3[:], t2[:], t1[:], op=ADD)
            o = sb.tile([128, S], bf16, tag=f"m1_{j}_{s}", name=f"m1_{j}_{s}")
            nc.scalar.activation(o[:], t3[:], AF.Silu, bias=fcb_t[:, j:j + 1])
            m1[j][s] = o

        for j2 in range(2):
            for s in range(NSEG):
                fc_finish(jp0_psums[(j2, s)], j2, s)
        for jp in range(1, 4):
            wb = wblock(fcw, jp)
            for j2 in range(2):
                for s in range(NSEG):
                    pm = fc_chain(wb, j2, s)
                    fc_finish(pm, jp * 2 + j2, s)

        # ---- MLP c_proj (H -> I) + bias + residual add
        oin = [[None] * NSEG for _ in range(IK)]
        for ip in range(8):
            wb = sb.tile([128, 2048], bf16, tag="cw2", bufs=2, name="cpwb")
            nc.scalar.dma_start(wb[:], cpw[:, ip * 2048:(ip + 1) * 2048])
            for i2 in range(2):
                i = ip * 2 + i2
                for s in range(NSEG):
                    pm = mm_ps()
                    for kk in range(HK):
                        nc.tensor.matmul(pm[:],
                                         wb[:, kk * 256 + i2 * 128:kk * 256 + i2 * 128 + 128],
                                         m1[kk][s][:],
                                         start=(kk == 0), stop=(kk == HK - 1))
                    tb2 = sb.tile([128, S], bf16, tag="tb", bufs=3, name="tb2")
                    nc.scalar.activation(tb2[:], pm[:], AF.Identity,
                                         bias=cpb_t[:, i:i + 1])
                    oi = sb.tile([128, S], bf16, tag=f"oin{i}_{s}",
                                 name=f"oin{i}_{s}")
                    nc.vector.tensor_tensor(oi[:], tb2[:], res[i][s][:], op=ADD)
                    oin[i][s] = oi

        # ---- out_proj (I -> H)
        for jp in range(4):
            wb = wblock(outw, jp)
            for j2 in range(2):
                j = jp * 2 + j2
                for s in range(NSEG):
                    pm = mm_ps()
                    for kk in range(IK):
                        nc.tensor.matmul(pm[:],
                                         wb[:, kk * 256 + j2 * 128:kk * 256 + j2 * 128 + 128],
                                         oin[kk][s][:],
                                         start=(kk == 0), stop=(kk == IK - 1))
                    yo = sb.tile([128, S], f32, tag="yo", bufs=3, name="yo")
                    nc.vector.tensor_copy(yo[:], pm[:])
                    nc.scalar.dma_start(yT[j * 128:(j + 1) * 128, s * S:(s + 1) * S],
                                        yo[:])

    nc.compile()
    return nc


def _pack(inputs):
    bf = ml_dtypes.bfloat16
    f = lambda name: np.asarray(inputs[name], np.float32)
    hs = np.ascontiguousarray(f("hidden_states"))
    wT = np.ascontiguousarray(f("in_proj_w").T)                 # [H, 2I]
    winp = np.empty((H, 2 * I), np.float32)
    for g in range(G):
        winp[:, g * 512:g * 512 + 256] = wT[:, g * 256:(g + 1) * 256]
        winp[:, g * 512 + 256:(g + 1) * 512] = wT[:, I + g * 256:I + (g + 1) * 256]
    # block layouts: [128, nblocks*4096]; block g holds 8 lhsT chunk groups
    # [128, 512] = [h_m0, h_m1, gate_m0, gate_m1] so each phase-block is one
    # contiguous DMA
    winb = np.ascontiguousarray(
        winp.reshape(HK, 128, G, 512).transpose(1, 2, 0, 3)
        .reshape(128, G * 4096).astype(bf))
    # layernorm gamma/beta folded into c_fc (exact): silu((hn*g+b) @ W.T + c)
    # = silu(hn @ (W*g).T + (c + W @ b))
    fcw_eff = f("fc_w") * f("ln_g")[None, :]
    fcb_eff = f("fc_b") + f("fc_w") @ f("ln_b")
    sfc_col = np.ascontiguousarray(
        fcw_eff.sum(axis=1, dtype=np.float64).astype(np.float32)
        .reshape(HK, 128).T)
    # fc/out blocks keyed by jp (pair of 128-col output blocks): block jp =
    # [128, 16 kk * 256], chunk (kk, j2) at col kk*256 + j2*128
    fcwb = np.ascontiguousarray(
        fcw_eff.T.reshape(IK, 128, 4, 2, 128).transpose(1, 2, 0, 3, 4)
        .reshape(128, 4 * 4096).astype(bf))
    outwb = np.ascontiguousarray(
        f("out_w").T.reshape(IK, 128, 4, 2, 128).transpose(1, 2, 0, 3, 4)
        .reshape(128, 4 * 4096).astype(bf))
    # cproj blocks keyed by ip: block ip = [128, 8 kk * 256]
    cpwb = np.ascontiguousarray(
        f("cproj_w").T.reshape(HK, 128, 8, 2, 128).transpose(1, 2, 0, 3, 4)
        .reshape(128, 8 * 2048).astype(bf))
    v = f("conv_w").reshape(G, 256, 2, 128, CK)                 # [g, j, cc, i, k]
    cwp = np.ascontiguousarray(
        v.transpose(3, 0, 2, 4, 1).reshape(128, G * 2048).astype(bf))
    shared = dict(
        win=winb, cw=cwp, fcw=fcwb, cpw=cpwb, outw=outwb,
        cbcol=np.ascontiguousarray(f("conv_b").reshape(IK, 128).T),
        cpbcol=np.ascontiguousarray(f("cproj_b").reshape(IK, 128).T),
        fcbcol=np.ascontiguousarray(fcb_eff.reshape(HK, 128).T),
        sfccol=sfc_col,
        onesf=np.ones((128, 128), np.float32),
    )
    ipw_h = f("in_proj_w")[:I]                                  # [I, H]
    in_maps = []
    for c in range(NCORES):
        b, q = divmod(c, QC)
        own = hs[b, q * T:(q + 1) * T]                          # [T, H]
        prev = (np.zeros((3, H), np.float32) if q == 0
                else hs[b, q * T - 3:q * T])
        xTc = np.ascontiguousarray(
            np.concatenate([prev, own], 0).T.astype(bf))        # [H, T+3]
        hh = np.zeros((IK, 128, 4), np.float32)
        hh[:, :, 0:3] = (ipw_h @ prev.T).reshape(IK, 128, 3)    # halo h columns
        hh = np.ascontiguousarray(
            hh.transpose(1, 0, 2).reshape(128, IK * 4).astype(bf))
        in_maps.append(dict(xT=xTc, haloh=hh, **shared))
    return in_maps


def _run(inputs, trace=False):
    from concourse.bass_utils import run_bass_kernel_spmd

    nc = _CACHE.get("nc")
    if nc is None:
        nc = _build()
        _CACHE["nc"] = nc
    in_maps = _pack(inputs)
    try:
        res = run_bass_kernel_spmd(nc, in_maps, core_ids=list(range(NCORES)),
                                   trace=trace)
    except Exception:
        # transient NRT_EXEC_UNIT_UNRECOVERABLE has been observed once after a
        # wedged prior run; one retry has always succeeded
        res = run_bass_kernel_spmd(nc, in_maps, core_ids=list(range(NCORES)),
                                   trace=trace)
    y = np.empty((B, L, H), np.float32)
    for c in range(NCORES):
        b, q = divmod(c, QC)
        y[b, q * T:(q + 1) * T, :] = res.results[c]["yT"].T
    return y, res


def kernel(**inputs) -> np.ndarray:
    y, _ = _run(inputs, trace=False)
    return y


# revision 26
# speedup vs baseline: 1.0225x; 1.0050x over previous
"""Trainium2 Bass kernel for nn_MetaMixer_6717328851330.

Computation (see reference):
    p = x @ W_in.T ; h, gate = split(p) ; gate = silu(gate)
    h = causal_grouped_conv1d(h) + b_conv ; h = h * gate       (residual)
    hn = layernorm_I(h) ; m = silu(hn @ W_fc.T + b_fc) @ W_cp.T + b_cp
    y = (m + residual) @ W_out.T

Strategy: pure data-parallel over the 8192 tokens (B*L), 1024 tokens per
core, no collectives. The causal-conv left halo (3 tokens) is shipped
pre-computed from the host (haloh); the intra-core segment halo comes from
the previous segment's h tiles resident in SBUF.

All matmuls run in bf16 (tolerance is 2e-2; measured bf16 error ~5e-3),
which halves HBM weight traffic AND halves the PE's per-matmul stationary
weight-load time vs fp32r (2-byte vs 4-byte loads). Each weight block is
DMA'd once and consumed by both 512-token segments back-to-back. Bias adds
and LN statistics run on the scalar/vector engines instead of K=1 matmuls,
which on HW cost a full 512-row pass each on the PE (the bottleneck
engine: it runs at ~95% occupancy).

On-core layout: activations live as [channel, token] tiles so every matmul
contracts along SBUF partitions with no transposes (host pre-transposes x
and all weights). PSUM fp32 accumulation throughout; LN stats in fp32.
"""
import sys

sys.path.insert(0, "/opt/trn_rl_repo")
import ml_dtypes
import numpy as np

NCORES = 8
B, L, H, I, G, CK = 2, 4096, 1024, 2048, 8, 4
T = (B * L) // NCORES          # tokens per core
S = 512                        # token segment (= psum bank free dim)
NSEG = T // S
HK = H // 128                  # 8  k-chunks over hidden
IK = I // 128                  # 16 k-chunks over intermediate
QC = NCORES // B               # seq chunks per batch
EPS = 1e-5

_CACHE = {}


def _build():
    import concourse.bacc as bacc
    import concourse.mybir as mybir
    import concourse.tile as tile
    from concourse.alu_op_type import AluOpType

    f32 = mybir.dt.float32
    f32r = mybir.dt.float32r
    bf16 = mybir.dt.bfloat16
    AF = mybir.ActivationFunctionType
    MUL, ADD, SUB = AluOpType.mult, AluOpType.add, AluOpType.subtract

    nc = bacc.Bacc(None, target_bir_lowering=False)

    xT = nc.dram_tensor("xT", [H, T + 3], bf16, kind="ExternalInput")
    win = nc.dram_tensor("win", [128, G * 4096], bf16, kind="ExternalInput")
    cw = nc.dram_tensor("cw", [128, G * 2048], bf16, kind="ExternalInput")
    fcw = nc.dram_tensor("fcw", [128, 4 * 4096], bf16, kind="ExternalInput")
    cpw = nc.dram_tensor("cpw", [128, 8 * 2048], bf16, kind="ExternalInput")
    outw = nc.dram_tensor("outw", [128, 4 * 4096], bf16, kind="ExternalInput")
    haloh = nc.dram_tensor("haloh", [128, IK * 4], bf16, kind="ExternalInput")
    cbcol = nc.dram_tensor("cbcol", [128, IK], f32, kind="ExternalInput")
    cpbcol = nc.dram_tensor("cpbcol", [128, IK], f32, kind="ExternalInput")
    fcbcol = nc.dram_tensor("fcbcol", [128, HK], f32, kind="ExternalInput")
    sfccol = nc.dram_tensor("sfccol", [128, HK], f32, kind="ExternalInput")
    onesf = nc.dram_tensor("onesf", [128, 128], f32r, kind="ExternalInput")
    yT = nc.dram_tensor("yT", [H, T], f32, kind="ExternalOutput")

    with nc.allow_low_precision(reason="bf16 matmul pipeline"), \
         tile.TileContext(nc) as tc, \
         tc.tile_pool(name="sb", bufs=1) as sb, \
         tc.tile_pool(name="ps", bufs=1, space="PSUM") as ps:

        def mm_ps():
            return ps.tile([128, S], f32, tag="mm", bufs=5, name="mmps")

        # ---- inputs + constants. Segment-0 x slices first: the first
        # in_proj chain only needs those 1MB, not the full 2.1MB of x.
        ones128 = sb.tile([1, 128], f32r, tag="ones128", name="ones128")
        nc.sync.dma_start(ones128[:], onesf[0:1, :])
        # 8 x tiles split over two DMA rings: each dma_start costs ~600ns of
        # its sequencer (DIRECT2D descriptor gen), so one ring would take
        # ~5us just to issue them
        xk = []
        for k in range(HK):
            t = sb.tile([128, T + 3], bf16, tag=f"x{k}", name=f"x{k}")
            eng = nc.scalar if k % 2 == 0 else nc.gpsimd
            eng.dma_start(t[:], xT[k * 128:(k + 1) * 128, :])
            xk.append(t)
        carryall = sb.tile([128, IK * 4], bf16, tag="carryall", name="carryall")
        nc.gpsimd.dma_start(carryall[:], haloh[:])
        ones_col = sb.tile([128, 1], f32r, tag="ones_col", name="ones_col")
        nc.gpsimd.dma_start(ones_col[:], onesf[:, 0:1])
        # p-state warm-up: ~dozen throwaway matmuls on the ones tile so the
        # PE clock is ramped when the first real chain's operands land
        warm = ps.tile([128, S], f32, tag="pba", bufs=3, name="warmps")
        for w in range(10):
            nc.tensor.matmul(warm[:, 0:128], ones128[:], ones128[:],
                             start=(w == 0), stop=(w == 9))
        cb_t = sb.tile([128, IK], f32, tag="cbt", name="cbt")
        nc.gpsimd.dma_start(cb_t[:], cbcol[:])
        cpb_t = sb.tile([128, IK], f32, tag="cpbt", name="cpbt")
        nc.gpsimd.dma_start(cpb_t[:], cpbcol[:])
        fcb_t = sb.tile([128, HK], f32, tag="fcbt", name="fcbt")
        nc.gpsimd.dma_start(fcb_t[:], fcbcol[:])
        sfc_t = sb.tile([128, HK], f32, tag="sfct", name="sfct")
        nc.gpsimd.dma_start(sfc_t[:], sfccol[:])

        res = [[None] * NSEG for _ in range(IK)]
        sumt = [None] * NSEG
        sqsum = [None] * NSEG

        # ---- Phase A: in_proj + causal grouped conv + silu-gate, per group
        for g in range(G):
            wg = sb.tile([128, 4096], bf16, tag="wbig", bufs=3, name="wbig")
            if g == 0:
                # fast start: 8 chunk-sized DMAs so the first matmul only
                # waits on 128KB, not the full 1MB block
                for k in range(HK):
                    nc.sync.dma_start(wg[:, k * 512:(k + 1) * 512],
                                      win[:, k * 512:(k + 1) * 512])
            else:
                # half-block DMAs: the chain's first matmuls start when the
                # first 512KB lands
                nc.sync.dma_start(wg[:, 0:2048], win[:, g * 4096:g * 4096 + 2048])
                nc.sync.dma_start(wg[:, 2048:4096],
                                  win[:, g * 4096 + 2048:(g + 1) * 4096])
            cwt = sb.tile([128, 2048], bf16, tag="cw2", bufs=2, name="cwt")
            nc.scalar.dma_start(cwt[:], cw[:, g * 2048:(g + 1) * 2048])

            hts = [[None] * NSEG, [None] * NSEG]
            for m in range(2):
                i2 = 2 * g + m
                for s in range(NSEG):
                    pm = mm_ps()
                    for k in range(HK):
                        nc.tensor.matmul(pm[:],
                                         wg[:, k * 512 + m * 128:k * 512 + m * 128 + 128],
                                         xk[k][:, 3 + s * S:3 + (s + 1) * S],
                                         start=(k == 0), stop=(k == HK - 1))
                    ht = sb.tile([128, S + 3], bf16, tag="hT", bufs=5, name="hT")
                    nc.vector.tensor_copy(ht[:, 3:S + 3], pm[:])
                    if s == 0:
                        nc.vector.tensor_copy(ht[:, 0:3],
                                              carryall[:, i2 * 4:i2 * 4 + 3])
                    else:
                        nc.vector.tensor_copy(ht[:, 0:3],
                                              hts[m][s - 1][:, S:S + 3])
                    hts[m][s] = ht

            gss = [[None] * NSEG, [None] * NSEG]
            for m in range(2):
                for s in range(NSEG):
                    pg = mm_ps()
                    for k in range(HK):
                        nc.tensor.matmul(pg[:],
                                         wg[:, k * 512 + 256 + m * 128:k * 512 + 384 + m * 128],
                                         xk[k][:, 3 + s * S:3 + (s + 1) * S],
                                         start=(k == 0), stop=(k == HK - 1))
                    gs = sb.tile([128, S], bf16, tag="gsc", bufs=4, name="gsc")
                    nc.scalar.activation(gs[:], pg[:], AF.Silu)
                    gss[m][s] = gs

            for m in range(2):
                i2 = 2 * g + m
                for s in range(NSEG):
                    pc = mm_ps()
                    first = True
                    for cc in range(2):
                        for k in range(CK):
                            c0 = cc * 1024 + k * 256 + m * 128
                            nc.tensor.matmul(pc[:], cwt[:, c0:c0 + 128],
                                             hts[cc][s][:, k:k + S],
                                             start=first,
                                             stop=(cc == 1 and k == CK - 1))
                            first = False
                    tb = sb.tile([128, S], bf16, tag="tb", bufs=3, name="tb")
                    nc.scalar.activation(tb[:], pc[:], AF.Identity,
                                         bias=cb_t[:, i2:i2 + 1])
                    rs = sb.tile([128, S], bf16, tag=f"res{i2}_{s}",
                                 name=f"res{i2}_{s}")
                    nc.vector.tensor_tensor(rs[:], tb[:], gss[m][s][:], op=MUL)
                    res[i2][s] = rs

            # ---- LN statistics, part 1: running channel sums / sum-squares
            # interleaved with phase A (cross-partition reduction finishes
            # with a single ones-matmul each below; doing all 16 k-chunks as
            # ones-matmuls would cost 60 more PE passes on the bottleneck)
            for s in range(NSEG):
                r0, r1 = res[2 * g][s], res[2 * g + 1][s]
                q0 = sb.tile([128, S], f32r, tag="sq", bufs=3, name="sq")
                nc.scalar.activation(q0[:], r0[:], AF.Square)
                q1 = sb.tile([128, S], f32r, tag="sq", bufs=3, name="sq")
                nc.scalar.activation(q1[:], r1[:], AF.Square)
                if g == 0:
                    a = sb.tile([128, S], f32r, tag=f"accs{s}", bufs=2,
                                name="accs")
                    nc.vector.tensor_tensor(a[:], r0[:], r1[:], op=ADD)
                    sumt[s] = a
                    aq = sb.tile([128, S], f32r, tag=f"accq{s}", bufs=2,
                                 name="accq")
                    nc.vector.tensor_tensor(aq[:], q0[:], q1[:], op=ADD)
                    sqsum[s] = aq
                else:
                    a1 = sb.tile([128, S], f32r, tag=f"accs{s}", bufs=2,
                                 name="accs")
                    nc.vector.tensor_tensor(a1[:], sumt[s][:], r0[:], op=ADD)
                    a2 = sb.tile([128, S], f32r, tag=f"accs{s}", bufs=2,
                                 name="accs")
                    nc.vector.tensor_tensor(a2[:], a1[:], r1[:], op=ADD)
                    sumt[s] = a2
                    aq1 = sb.tile([128, S], f32r, tag=f"accq{s}", bufs=2,
                                  name="accq")
                    nc.vector.tensor_tensor(aq1[:], sqsum[s][:], q0[:], op=ADD)
                    aq2 = sb.tile([128, S], f32r, tag=f"accq{s}", bufs=2,
                                  name="accq")
                    nc.vector.tensor_tensor(aq2[:], aq1[:], q1[:], op=ADD)
                    sqsum[s] = aq2

        # ---- MLP c_fc, chains only for the first H-block: keeps the PE busy
        # while the LN row math (below) runs on vector/scalar
        def fc_chain(wb, j2, s):
            pm = mm_ps()
            for kk in range(IK):
                nc.tensor.matmul(pm[:],
                                 wb[:, kk * 256 + j2 * 128:kk * 256 + j2 * 128 + 128],
                                 res[kk][s][:],
                                 start=(kk == 0), stop=(kk == IK - 1))
            return pm

        def wblock(src, jp):
            wb = sb.tile([128, 4096], bf16, tag="wbig", bufs=3, name="wbig2")
            nc.sync.dma_start(wb[:, 0:2048], src[:, jp * 4096:jp * 4096 + 2048])
            nc.sync.dma_start(wb[:, 2048:4096],
                              src[:, jp * 4096 + 2048:(jp + 1) * 4096])
            return wb

        fcwb0 = wblock(fcw, 0)
        jp0_psums = {}
        jp0_psums[(0, 0)] = fc_chain(fcwb0, 0, 0)

        # ---- LN statistics, part 2: finish stats. The PE reaches the stat
        # matmuls ~8us into the jp0 chains above, by which time the running
        # accumulators have drained; the j2=1 chains below then cover the
        # serial row-math latency (incl. the Sqrt act-table load) before the
        # pba broadcasts need arow/brow.
        bcA = [None] * NSEG
        bcB = [None] * NSEG
        stat_ps = {}

        def emit_stats(s):
            pst0 = ps.tile([1, S], f32, tag="pba", bufs=3, name="statps")
            nc.tensor.matmul(pst0[:], ones_col[:], sumt[s][:],
                             start=True, stop=True)
            pst1 = ps.tile([1, S], f32, tag="pba", bufs=3, name="statps")
            nc.tensor.matmul(pst1[:], ones_col[:], sqsum[s][:],
                             start=True, stop=True)
            stat_ps[s] = (pst0, pst1)

        emit_stats(0)
        jp0_psums[(0, 1)] = fc_chain(fcwb0, 0, 1)
        emit_stats(1)
        # row math level-by-level across both segments: the two Sqrts stay
        # adjacent on the scalar engine (one act-table load, not two) and the
        # reciprocal uses the fast DVE approximation (exact reciprocal() is
        # 3.3us per row and sat on the critical path)
        rows = {}
        mnegs, msqs, vrows, sds = {}, {}, {}, {}
        for s in range(NSEG):
            mneg = sb.tile([1, S], f32r, tag="mneg", bufs=2, name="mneg")
            nc.vector.tensor_scalar(mneg[:], stat_ps[s][0][:], -1.0 / I,
                                    None, op0=MUL)
            mnegs[s] = mneg
        for s in range(NSEG):
            msq = sb.tile([1, S], f32, tag="msq", bufs=2, name="msq")
            nc.scalar.activation(msq[:], stat_ps[s][0][:], AF.Square)
            msqs[s] = msq
        for s in range(NSEG):
            msq = msqs[s]
            nc.vector.tensor_scalar(msq[:], msq[:], 1.0 / I, None, op0=MUL)
            vrow = sb.tile([1, S], f32, tag="vrow", bufs=2, name="vrow")
            nc.vector.tensor_tensor(vrow[:], stat_ps[s][1][:], msq[:], op=SUB)
            nc.vector.tensor_scalar(vrow[:], vrow[:], 1.0 / I, EPS,
                                    op0=MUL, op1=ADD)
            vrows[s] = vrow
        for s in range(NSEG):
            sd = sb.tile([1, S], f32, tag="sd", bufs=2, name="sd")
            nc.scalar.activation(sd[:], vrows[s][:], AF.Sqrt)
            sds[s] = sd
        for s in range(NSEG):
            ar32 = sb.tile([1, S], f32, tag="ar32", bufs=2, name="ar32")
            nc.vector.reciprocal_approx_fast(ar32[:], sds[s][:])    # rstd
            arow = sb.tile([1, S], f32r, tag="arow", bufs=2, name="arow")
            nc.vector.tensor_copy(arow[:], ar32[:])
            brow = sb.tile([1, S], f32r, tag="brow", bufs=2, name="brow")
            nc.vector.tensor_tensor(brow[:], mnegs[s][:], arow[:], op=MUL)
            rows[s] = (arow, brow)
        for s in range(NSEG):
            jp0_psums[(1, s)] = fc_chain(fcwb0, 1, s)
        for s in range(NSEG):
            arow, brow = rows[s]
            pa = ps.tile([128, S], f32, tag="pba", bufs=3, name="pbaps")
            nc.tensor.matmul(pa[:], ones128[:], arow[:], start=True, stop=True)
            bA = sb.tile([128, S], bf16, tag=f"bcA{s}", name=f"bcA{s}")
            nc.vector.tensor_copy(bA[:], pa[:])
            bcA[s] = bA
            pb = ps.tile([128, S], f32, tag="pba", bufs=3, name="pbaps")
            nc.tensor.matmul(pb[:], ones128[:], brow[:], start=True, stop=True)
            bB = sb.tile([128, S], bf16, tag=f"bcB{s}", name=f"bcB{s}")
            nc.vector.tensor_copy(bB[:], pb[:])
            bcB[s] = bB

        # ---- MLP c_fc: hn = (psum - mean*S_fc_row)*rstd, done per tile as
        # psum*bcA + S_fc[j]*bcB, then silu(+folded bias)
        m1 = [[None] * NSEG for _ in range(HK)]

        def fc_finish(pm, j, s):
            t1 = sb.tile([128, S], bf16, tag="fct1", bufs=2, name="fct1")
            nc.vector.tensor_scalar(t1[:], bcB[s][:], sfc_t[:, j:j + 1], None,
                                    op0=MUL)
            t2 = sb.tile([128, S], bf16, tag="fct2", bufs=2, name="fct2")
            nc.vector.tensor_tensor(t2[:], pm[:], bcA[s][:], op=MUL)
            t3 = sb.tile([128, S], bf16, tag="fct3", bufs=2, name="fct3")
            nc.vector.tensor_tensor(t3[:], t2[:], t1[:], op=ADD)
            o = sb.tile([128, S], bf16, tag=f"m1_{j}_{s}", name=f"m1_{j}_{s}")
            nc.scalar.activation(o[:], t3[:], AF.Silu, bias=fcb_t[:, j:j + 1])
            m1[j][s] = o

        for j2 in range(2):
            for s in range(NSEG):
                fc_finish(jp0_psums[(j2, s)], j2, s)
        for jp in range(1, 4):
            wb = wblock(fcw, jp)
            for j2 in range(2):
                for s in range(NSEG):
                    pm = fc_chain(wb, j2, s)
                    fc_finish(pm, jp * 2 + j2, s)

        # ---- MLP c_proj (H -> I) + bias + residual add
        oin = [[None] * NSEG for _ in range(IK)]
        for ip in range(8):
            wb = sb.tile([128, 2048], bf16, tag="cw2", bufs=2, name="cpwb")
            nc.scalar.dma_start(wb[:], cpw[:, ip * 2048:(ip + 1) * 2048])
            for i2 in range(2):
                i = ip * 2 + i2
                for s in range(NSEG):
                    pm = mm_ps()
                    for kk in range(HK):
                        nc.tensor.matmul(pm[:],
                                         wb[:, kk * 256 + i2 * 128:kk * 256 + i2 * 128 + 128],
                                         m1[kk][s][:],
                                         start=(kk == 0), stop=(kk == HK - 1))
                    tb2 = sb.tile([128, S], bf16, tag="tb", bufs=3, name="tb2")
                    nc.scalar.activation(tb2[:], pm[:], AF.Identity,
                                         bias=cpb_t[:, i:i + 1])
                    oi = sb.tile([128, S], bf16, tag=f"oin{i}_{s}",
                                 name=f"oin{i}_{s}")
                    nc.vector.tensor_tensor(oi[:], tb2[:], res[i][s][:], op=ADD)
                    oin[i][s] = oi

        # ---- out_proj (I -> H)
        for jp in range(4):
            wb = wblock(outw, jp)
            for j2 in range(2):
                j = jp * 2 + j2
                for s in range(NSEG):
                    pm = mm_ps()
                    for kk in range(IK):
                        nc.tensor.matmul(pm[:],
                                         wb[:, kk * 256 + j2 * 128:kk * 256 + j2 * 128 + 128],
                                         oin[kk][s][:],
                                         start=(kk == 0), stop=(kk == IK - 1))
                    yo = sb.tile([128, S], f32, tag="yo", bufs=3, name="yo")
                    nc.vector.tensor_copy(yo[:], pm[:])
                    nc.scalar.dma_start(yT[j * 128:(j + 1) * 128, s * S:(s + 1) * S],
                                        yo[:])

    nc.compile()
    return nc


def _pack(inputs):
    bf = ml_dtypes.bfloat16
    f = lambda name: np.asarray(inputs[name], np.float32)
    hs = np.ascontiguousarray(f("hidden_states"))
    wT = np.ascontiguousarray(f("in_proj_w").T)                 # [H, 2I]
    winp = np.empty((H, 2 * I), np.float32)
    for g in range(G):
        winp[:, g * 512:g * 512 + 256] = wT[:, g * 256:(g + 1) * 256]
        winp[:, g * 512 + 256:(g + 1) * 512] = wT[:, I + g * 256:I + (g + 1) * 256]
    # block layouts: [128, nblocks*4096]; block g holds 8 lhsT chunk groups
    # [128, 512] = [h_m0, h_m1, gate_m0, gate_m1] so each phase-block is one
    # contiguous DMA
    winb = np.ascontiguousarray(
        winp.reshape(HK, 128, G, 512).transpose(1, 2, 0, 3)
        .reshape(128, G * 4096).astype(bf))
    # layernorm gamma/beta folded into c_fc (exact): silu((hn*g+b) @ W.T + c)
    # = silu(hn @ (W*g).T + (c + W @ b))
    fcw_eff = f("fc_w") * f("ln_g")[None, :]
    fcb_eff = f("fc_b") + f("fc_w") @ f("ln_b")
    sfc_col = np.ascontiguousarray(
        fcw_eff.sum(axis=1, dtype=np.float64).astype(np.float32)
        .reshape(HK, 128).T)
    # fc/out blocks keyed by jp (pair of 128-col output blocks): block jp =
    # [128, 16 kk * 256], chunk (kk, j2) at col kk*256 + j2*128
    fcwb = np.ascontiguousarray(
        fcw_eff.T.reshape(IK, 128, 4, 2, 128).transpose(1, 2, 0, 3, 4)
        .reshape(128, 4 * 4096).astype(bf))
    outwb = np.ascontiguousarray(
        f("out_w").T.reshape(IK, 128, 4, 2, 128).transpose(1, 2, 0, 3, 4)
        .reshape(128, 4 * 4096).astype(bf))
    # cproj blocks keyed by ip: block ip = [128, 8 kk * 256]
    cpwb = np.ascontiguousarray(
        f("cproj_w").T.reshape(HK, 128, 8, 2, 128).transpose(1, 2, 0, 3, 4)
        .reshape(128, 8 * 2048).astype(bf))
    v = f("conv_w").reshape(G, 256, 2, 128, CK)                 # [g, j, cc, i, k]
    cwp = np.ascontiguousarray(
        v.transpose(3, 0, 2, 4, 1).reshape(128, G * 2048).astype(bf))
    shared = dict(
        win=winb, cw=cwp, fcw=fcwb, cpw=cpwb, outw=outwb,
        cbcol=np.ascontiguousarray(f("conv_b").reshape(IK, 128).T),
        cpbcol=np.ascontiguousarray(f("cproj_b").reshape(IK, 128).T),
        fcbcol=np.ascontiguousarray(fcb_eff.reshape(HK, 128).T),
        sfccol=sfc_col,
        onesf=np.ones((128, 128), np.float32),
    )
    ipw_h = f("in_proj_w")[:I]                                  # [I, H]
    in_maps = []
    for c in range(NCORES):
        b, q = divmod(c, QC)
        own = hs[b, q * T:(q + 1) * T]                          # [T, H]
        prev = (np.zeros((3, H), np.float32) if q == 0
                else hs[b, q * T - 3:q * T])
        xTc = np.ascontiguousarray(
            np.concatenate([prev, own], 0).T.astype(bf))        # [H, T+3]
        hh = np.zeros((IK, 128, 4), np.float32)
        hh[:, :, 0:3] = (ipw_h @ prev.T).reshape(IK, 128, 3)    # halo h columns
        hh = np.ascontiguousarray(
            hh.transpose(1, 0, 2).reshape(128, IK * 4).astype(bf))
        in_maps.append(dict(xT=xTc, haloh=hh, **shared))
    return in_maps


def _run(inputs, trace=False):
    from concourse.bass_utils import run_bass_kernel_spmd

    nc = _CACHE.get("nc")
    if nc is None:
        nc = _build()
        _CACHE["nc"] = nc
    in_maps = _pack(inputs)
    try:
        res = run_bass_kernel_spmd(nc, in_maps, core_ids=list(range(NCORES)),
                                   trace=trace)
    except Exception:
        # transient NRT_EXEC_UNIT_UNRECOVERABLE has been observed once after a
        # wedged prior run; one retry has always succeeded
        res = run_bass_kernel_spmd(nc, in_maps, core_ids=list(range(NCORES)),
                                   trace=trace)
    y = np.empty((B, L, H), np.float32)
    for c in range(NCORES):
        b, q = divmod(c, QC)
        y[b, q * T:(q + 1) * T, :] = res.results[c]["yT"].T
    return y, res


def kernel(**inputs) -> np.ndarray:
    y, _ = _run(inputs, trace=False)
    return y


# revision 29
# speedup vs baseline: 1.0241x; 1.0016x over previous
"""Trainium2 Bass kernel for nn_MetaMixer_6717328851330.

Computation (see reference):
    p = x @ W_in.T ; h, gate = split(p) ; gate = silu(gate)
    h = causal_grouped_conv1d(h) + b_conv ; h = h * gate       (residual)
    hn = layernorm_I(h) ; m = silu(hn @ W_fc.T + b_fc) @ W_cp.T + b_cp
    y = (m + residual) @ W_out.T

Strategy: pure data-parallel over the 8192 tokens (B*L), 1024 tokens per
core, no collectives. The causal-conv left halo (3 tokens) is shipped
pre-computed from the host (haloh); the intra-core segment halo comes from
the previous segment's h tiles resident in SBUF.

All matmuls run in bf16 (tolerance is 2e-2; measured bf16 error ~5e-3),
which halves HBM weight traffic AND halves the PE's per-matmul stationary
weight-load time vs fp32r (2-byte vs 4-byte loads). Each weight block is
DMA'd once and consumed by both 512-token segments back-to-back. Bias adds
and LN statistics run on the scalar/vector engines instead of K=1 matmuls,
which on HW cost a full 512-row pass each on the PE (the bottleneck
engine: it runs at ~95% occupancy).

On-core layout: activations live as [channel, token] tiles so every matmul
contracts along SBUF partitions with no transposes (host pre-transposes x
and all weights). PSUM fp32 accumulation throughout; LN stats in fp32.
"""
import sys

sys.path.insert(0, "/opt/trn_rl_repo")
import ml_dtypes
import numpy as np

NCORES = 8
B, L, H, I, G, CK = 2, 4096, 1024, 2048, 8, 4
T = (B * L) // NCORES          # tokens per core
S = 512                        # token segment (= psum bank free dim)
NSEG = T // S
HK = H // 128                  # 8  k-chunks over hidden
IK = I // 128                  # 16 k-chunks over intermediate
QC = NCORES // B               # seq chunks per batch
EPS = 1e-5

_CACHE = {}


def _build():
    import concourse.bacc as bacc
    import concourse.mybir as mybir
    import concourse.tile as tile
    from concourse.alu_op_type import AluOpType

    f32 = mybir.dt.float32
    f32r = mybir.dt.float32r
    bf16 = mybir.dt.bfloat16
    AF = mybir.ActivationFunctionType
    MUL, ADD, SUB = AluOpType.mult, AluOpType.add, AluOpType.subtract

    nc = bacc.Bacc(None, target_bir_lowering=False)

    xT = nc.dram_tensor("xT", [H, T + 3], bf16, kind="ExternalInput")
    win = nc.dram_tensor("win", [128, G * 4096], bf16, kind="ExternalInput")
    cw = nc.dram_tensor("cw", [128, G * 2048], bf16, kind="ExternalInput")
    fcw = nc.dram_tensor("fcw", [128, 4 * 4096], bf16, kind="ExternalInput")
    cpw = nc.dram_tensor("cpw", [128, 8 * 2048], bf16, kind="ExternalInput")
    outw = nc.dram_tensor("outw", [128, 4 * 4096], bf16, kind="ExternalInput")
    haloh = nc.dram_tensor("haloh", [128, IK * 4], bf16, kind="ExternalInput")
    cbcol = nc.dram_tensor("cbcol", [128, IK], f32, kind="ExternalInput")
    cpbcol = nc.dram_tensor("cpbcol", [128, IK], f32, kind="ExternalInput")
    fcbcol = nc.dram_tensor("fcbcol", [128, HK], f32, kind="ExternalInput")
    sfccol = nc.dram_tensor("sfccol", [128, HK], f32, kind="ExternalInput")
    onesf = nc.dram_tensor("onesf", [128, 128], f32r, kind="ExternalInput")
    yT = nc.dram_tensor("yT", [H, T], f32, kind="ExternalOutput")

    with nc.allow_low_precision(reason="bf16 matmul pipeline"), \
         tile.TileContext(nc) as tc, \
         tc.tile_pool(name="sb", bufs=1) as sb, \
         tc.tile_pool(name="ps", bufs=1, space="PSUM") as ps:

        def mm_ps():
            return ps.tile([128, S], f32, tag="mm", bufs=5, name="mmps")

        # ---- inputs + constants. Segment-0 x slices first: the first
        # in_proj chain only needs those 1MB, not the full 2.1MB of x.
        ones128 = sb.tile([1, 128], f32r, tag="ones128", name="ones128")
        nc.sync.dma_start(ones128[:], onesf[0:1, :])
        # x split per segment and over two DMA rings (each dma_start costs
        # ~600ns of its sequencer), segment-0 tiles first: phase A runs
        # segment-major, so the first ~10us of PE work needs only seg-0 x
        xk = [[None] * HK for _ in range(NSEG)]
        for s in range(NSEG):
            for k in range(HK):
                t = sb.tile([128, S + 3], bf16, tag=f"x{s}_{k}",
                            name=f"x{s}_{k}")
                eng = nc.scalar if k % 2 == 0 else nc.gpsimd
                eng.dma_start(t[:], xT[k * 128:(k + 1) * 128,
                                       s * S:s * S + S + 3])
                xk[s][k] = t
        carryall = sb.tile([128, IK * 4], bf16, tag="carryall", name="carryall")
        nc.gpsimd.dma_start(carryall[:], haloh[:])
        ones_col = sb.tile([128, 1], f32r, tag="ones_col", name="ones_col")
        nc.gpsimd.dma_start(ones_col[:], onesf[:, 0:1])
        # p-state warm-up: ~dozen throwaway matmuls on the ones tile so the
        # PE clock is ramped when the first real chain's operands land
        warm = ps.tile([128, S], f32, tag="pba", bufs=3, name="warmps")
        for w in range(10):
            nc.tensor.matmul(warm[:, 0:128], ones128[:], ones128[:],
                             start=(w == 0), stop=(w == 9))
        cb_t = sb.tile([128, IK], f32, tag="cbt", name="cbt")
        nc.gpsimd.dma_start(cb_t[:], cbcol[:])
        cpb_t = sb.tile([128, IK], f32, tag="cpbt", name="cpbt")
        nc.gpsimd.dma_start(cpb_t[:], cpbcol[:])
        fcb_t = sb.tile([128, HK], f32, tag="fcbt", name="fcbt")
        nc.gpsimd.dma_start(fcb_t[:], fcbcol[:])
        sfc_t = sb.tile([128, HK], f32, tag="sfct", name="sfct")
        nc.gpsimd.dma_start(sfc_t[:], sfccol[:])

        res = [[None] * NSEG for _ in range(IK)]
        sumt = [None] * NSEG
        sqsum = [None] * NSEG

        # ---- Phase A: in_proj + causal grouped conv + silu-gate, per group
        for g in range(G):
            wg = sb.tile([128, 4096], bf16, tag="wbig", bufs=3, name="wbig")
            if g == 0:
                # fast start: 8 chunk-sized DMAs so the first matmul only
                # waits on 128KB, not the full 1MB block
                for k in range(HK):
                    nc.sync.dma_start(wg[:, k * 512:(k + 1) * 512],
                                      win[:, k * 512:(k + 1) * 512])
            else:
                # half-block DMAs: the chain's first matmuls start when the
                # first 512KB lands
                nc.sync.dma_start(wg[:, 0:2048], win[:, g * 4096:g * 4096 + 2048])
                nc.sync.dma_start(wg[:, 2048:4096],
                                  win[:, g * 4096 + 2048:(g + 1) * 4096])
            cwt = sb.tile([128, 2048], bf16, tag="cw2", bufs=2, name="cwt")
            nc.scalar.dma_start(cwt[:], cw[:, g * 2048:(g + 1) * 2048])

            # segment-major chain order: all of seg-0's h/gate/conv chains
            # run before seg-1's x tiles are needed
            hts = [[None] * NSEG, [None] * NSEG]
            gss = [[None] * NSEG, [None] * NSEG]
            for s in range(NSEG):
                for m in range(2):
                    i2 = 2 * g + m
                    pm = mm_ps()
                    for k in range(HK):
                        nc.tensor.matmul(pm[:],
                                         wg[:, k * 512 + m * 128:k * 512 + m * 128 + 128],
                                         xk[s][k][:, 3:3 + S],
                                         start=(k == 0), stop=(k == HK - 1))
                    ht = sb.tile([128, S + 3], bf16, tag="hT", bufs=5, name="hT")
                    nc.vector.tensor_copy(ht[:, 3:S + 3], pm[:])
                    if s == 0:
                        nc.vector.tensor_copy(ht[:, 0:3],
                                              carryall[:, i2 * 4:i2 * 4 + 3])
                    else:
                        nc.vector.tensor_copy(ht[:, 0:3],
                                              hts[m][s - 1][:, S:S + 3])
                    hts[m][s] = ht
                for m in range(2):
                    pg = mm_ps()
                    for k in range(HK):
                        nc.tensor.matmul(pg[:],
                                         wg[:, k * 512 + 256 + m * 128:k * 512 + 384 + m * 128],
                                         xk[s][k][:, 3:3 + S],
                                         start=(k == 0), stop=(k == HK - 1))
                    gs = sb.tile([128, S], bf16, tag=f"gsc{s}", bufs=2,
                                 name="gsc")
                    nc.scalar.activation(gs[:], pg[:], AF.Silu)
                    gss[m][s] = gs
                for m in range(2):
                    i2 = 2 * g + m
                    pc = mm_ps()
                    first = True
                    for cc in range(2):
                        for k in range(CK):
                            c0 = cc * 1024 + k * 256 + m * 128
                            nc.tensor.matmul(pc[:], cwt[:, c0:c0 + 128],
                                             hts[cc][s][:, k:k + S],
                                             start=first,
                                             stop=(cc == 1 and k == CK - 1))
                            first = False
                    tb = sb.tile([128, S], bf16, tag="tb", bufs=3, name="tb")
                    nc.scalar.activation(tb[:], pc[:], AF.Identity,
                                         bias=cb_t[:, i2:i2 + 1])
                    rs = sb.tile([128, S], bf16, tag=f"res{i2}_{s}",
                                 name=f"res{i2}_{s}")
                    nc.vector.tensor_tensor(rs[:], tb[:], gss[m][s][:], op=MUL)
                    res[i2][s] = rs

            # ---- LN statistics, part 1: running channel sums / sum-squares
            # interleaved with phase A (cross-partition reduction finishes
            # with a single ones-matmul each below; doing all 16 k-chunks as
            # ones-matmuls would cost 60 more PE passes on the bottleneck)
            for s in range(NSEG):
                r0, r1 = res[2 * g][s], res[2 * g + 1][s]
                q0 = sb.tile([128, S], f32r, tag="sq", bufs=3, name="sq")
                nc.scalar.activation(q0[:], r0[:], AF.Square)
                q1 = sb.tile([128, S], f32r, tag="sq", bufs=3, name="sq")
                nc.scalar.activation(q1[:], r1[:], AF.Square)
                if g == 0:
                    a = sb.tile([128, S], f32r, tag=f"accs{s}", bufs=2,
                                name="accs")
                    nc.vector.tensor_tensor(a[:], r0[:], r1[:], op=ADD)
                    sumt[s] = a
                    aq = sb.tile([128, S], f32r, tag=f"accq{s}", bufs=2,
                                 name="accq")
                    nc.vector.tensor_tensor(aq[:], q0[:], q1[:], op=ADD)
                    sqsum[s] = aq
                else:
                    a1 = sb.tile([128, S], f32r, tag=f"accs{s}", bufs=2,
                                 name="accs")
                    nc.vector.tensor_tensor(a1[:], sumt[s][:], r0[:], op=ADD)
                    a2 = sb.tile([128, S], f32r, tag=f"accs{s}", bufs=2,
                                 name="accs")
                    nc.vector.tensor_tensor(a2[:], a1[:], r1[:], op=ADD)
                    sumt[s] = a2
                    aq1 = sb.tile([128, S], f32r, tag=f"accq{s}", bufs=2,
                                  name="accq")
                    nc.vector.tensor_tensor(aq1[:], sqsum[s][:], q0[:], op=ADD)
                    aq2 = sb.tile([128, S], f32r, tag=f"accq{s}", bufs=2,
                                  name="accq")
                    nc.vector.tensor_tensor(aq2[:], aq1[:], q1[:], op=ADD)
                    sqsum[s] = aq2

        # ---- MLP c_fc, chains only for the first H-block: keeps the PE busy
        # while the LN row math (below) runs on vector/scalar
        def fc_chain(wb, j2, s):
            pm = mm_ps()
            for kk in range(IK):
                nc.tensor.matmul(pm[:],
                                 wb[:, kk * 256 + j2 * 128:kk * 256 + j2 * 128 + 128],
                                 res[kk][s][:],
                                 start=(kk == 0), stop=(kk == IK - 1))
            return pm

        def wblock(src, jp):
            wb = sb.tile([128, 4096], bf16, tag="wbig", bufs=3, name="wbig2")
            nc.sync.dma_start(wb[:, 0:2048], src[:, jp * 4096:jp * 4096 + 2048])
            nc.sync.dma_start(wb[:, 2048:4096],
                              src[:, jp * 4096 + 2048:(jp + 1) * 4096])
            return wb

        fcwb0 = wblock(fcw, 0)
        jp0_psums = {}
        jp0_psums[(0, 0)] = fc_chain(fcwb0, 0, 0)

        # ---- LN statistics, part 2: finish stats. The PE reaches the stat
        # matmuls ~8us into the jp0 chains above, by which time the running
        # accumulators have drained; the j2=1 chains below then cover the
        # serial row-math latency (incl. the Sqrt act-table load) before the
        # pba broadcasts need arow/brow.
        bcA = [None] * NSEG
        bcB = [None] * NSEG
        stat_ps = {}

        def emit_stats(s):
            pst0 = ps.tile([1, S], f32, tag="pba", bufs=3, name="statps")
            nc.tensor.matmul(pst0[:], ones_col[:], sumt[s][:],
                             start=True, stop=True)
            pst1 = ps.tile([1, S], f32, tag="pba", bufs=3, name="statps")
            nc.tensor.matmul(pst1[:], ones_col[:], sqsum[s][:],
                             start=True, stop=True)
            stat_ps[s] = (pst0, pst1)

        emit_stats(0)
        jp0_psums[(0, 1)] = fc_chain(fcwb0, 0, 1)
        emit_stats(1)
        # row math level-by-level across both segments: the two Sqrts stay
        # adjacent on the scalar engine (one act-table load, not two) and the
        # reciprocal uses the fast DVE approximation (exact reciprocal() is
        # 3.3us per row and sat on the critical path)
        rows = {}
        mnegs, msqs, vrows, sds = {}, {}, {}, {}
        for s in range(NSEG):
            mneg = sb.tile([1, S], f32r, tag="mneg", bufs=2, name="mneg")
            nc.vector.tensor_scalar(mneg[:], stat_ps[s][0][:], -1.0 / I,
                                    None, op0=MUL)
            mnegs[s] = mneg
        for s in range(NSEG):
            msq = sb.tile([1, S], f32, tag="msq", bufs=2, name="msq")
            nc.scalar.activation(msq[:], stat_ps[s][0][:], AF.Square)
            msqs[s] = msq
        for s in range(NSEG):
            msq = msqs[s]
            nc.vector.tensor_scalar(msq[:], msq[:], 1.0 / I, None, op0=MUL)
            vrow = sb.tile([1, S], f32, tag="vrow", bufs=2, name="vrow")
            nc.vector.tensor_tensor(vrow[:], stat_ps[s][1][:], msq[:], op=SUB)
            nc.vector.tensor_scalar(vrow[:], vrow[:], 1.0 / I, EPS,
                                    op0=MUL, op1=ADD)
            vrows[s] = vrow
        for s in range(NSEG):
            sd = sb.tile([1, S], f32, tag="sd", bufs=2, name="sd")
            nc.scalar.activation(sd[:], vrows[s][:], AF.Sqrt)
            sds[s] = sd
        for s in range(NSEG):
            ar32 = sb.tile([1, S], f32, tag="ar32", bufs=2, name="ar32")
            nc.vector.reciprocal_approx_fast(ar32[:], sds[s][:])    # rstd
            arow = sb.tile([1, S], f32r, tag="arow", bufs=2, name="arow")
            nc.vector.tensor_copy(arow[:], ar32[:])
            brow = sb.tile([1, S], f32r, tag="brow", bufs=2, name="brow")
            nc.vector.tensor_tensor(brow[:], mnegs[s][:], arow[:], op=MUL)
            rows[s] = (arow, brow)
        for s in range(NSEG):
            jp0_psums[(1, s)] = fc_chain(fcwb0, 1, s)
        for s in range(NSEG):
            arow, brow = rows[s]
            pa = ps.tile([128, S], f32, tag="pba", bufs=3, name="pbaps")
            nc.tensor.matmul(pa[:], ones128[:], arow[:], start=True, stop=True)
            bA = sb.tile([128, S], bf16, tag=f"bcA{s}", name=f"bcA{s}")
            nc.vector.tensor_copy(bA[:], pa[:])
            bcA[s] = bA
            pb = ps.tile([128, S], f32, tag="pba", bufs=3, name="pbaps")
            nc.tensor.matmul(pb[:], ones128[:], brow[:], start=True, stop=True)
            bB = sb.tile([128, S], bf16, tag=f"bcB{s}", name=f"bcB{s}")
            nc.vector.tensor_copy(bB[:], pb[:])
            bcB[s] = bB

        # ---- MLP c_fc: hn = (psum - mean*S_fc_row)*rstd, done per tile as
        # psum*bcA + S_fc[j]*bcB, then silu(+folded bias)
        m1 = [[None] * NSEG for _ in range(HK)]

        def fc_finish(pm, j, s):
            t1 = sb.tile([128, S], bf16, tag="fct1", bufs=2, name="fct1")
            nc.vector.tensor_scalar(t1[:], bcB[s][:], sfc_t[:, j:j + 1], None,
                                    op0=MUL)
            t2 = sb.tile([128, S], bf16, tag="fct2", bufs=2, name="fct2")
            nc.vector.tensor_tensor(t2[:], pm[:], bcA[s][:], op=MUL)
            t3 = sb.tile([128, S], bf16, tag="fct3", bufs=2, name="fct3")
            nc.vector.tensor_tensor(t3[:], t2[:], t1[:], op=ADD)
            o = sb.tile([128, S], bf16, tag=f"m1_{j}_{s}", name=f"m1_{j}_{s}")
            nc.scalar.activation(o[:], t3[:], AF.Silu, bias=fcb_t[:, j:j + 1])
            m1[j][s] = o

        for j2 in range(2):
            for s in range(NSEG):
                fc_finish(jp0_psums[(j2, s)], j2, s)
        for jp in range(1, 4):
            wb = wblock(fcw, jp)
            for j2 in range(2):
                for s in range(NSEG):
                    pm = fc_chain(wb, j2, s)
                    fc_finish(pm, jp * 2 + j2, s)

        # ---- MLP c_proj (H -> I) + bias + residual add
        oin = [[None] * NSEG for _ in range(IK)]
        for ip in range(8):
            wb = sb.tile([128, 2048], bf16, tag="cw2", bufs=2, name="cpwb")
            nc.scalar.dma_start(wb[:], cpw[:, ip * 2048:(ip + 1) * 2048])
            for i2 in range(2):
                i = ip * 2 + i2
                for s in range(NSEG):
                    pm = mm_ps()
                    for kk in range(HK):
                        nc.tensor.matmul(pm[:],
                                         wb[:, kk * 256 + i2 * 128:kk * 256 + i2 * 128 + 128],
                                         m1[kk][s][:],
                                         start=(kk == 0), stop=(kk == HK - 1))
                    tb2 = sb.tile([128, S], bf16, tag="tb", bufs=3, name="tb2")
                    nc.scalar.activation(tb2[:], pm[:], AF.Identity,
                                         bias=cpb_t[:, i:i + 1])
                    oi = sb.tile([128, S], bf16, tag=f"oin{i}_{s}",
                                 name=f"oin{i}_{s}")
                    nc.vector.tensor_tensor(oi[:], tb2[:], res[i][s][:], op=ADD)
                    oin[i][s] = oi

        # ---- out_proj (I -> H)
        for jp in range(4):
            wb = wblock(outw, jp)
            for j2 in range(2):
                j = jp * 2 + j2
                for s in range(NSEG):
                    pm = mm_ps()
                    for kk in range(IK):
                        nc.tensor.matmul(pm[:],
                                         wb[:, kk * 256 + j2 * 128:kk * 256 + j2 * 128 + 128],
                                         oin[kk][s][:],
                                         start=(kk == 0), stop=(kk == IK - 1))
                    yo = sb.tile([128, S], f32, tag="yo", bufs=3, name="yo")
                    nc.vector.tensor_copy(yo[:], pm[:])
                    nc.scalar.dma_start(yT[j * 128:(j + 1) * 128, s * S:(s + 1) * S],
                                        yo[:])

    nc.compile()
    return nc


def _pack(inputs):
    bf = ml_dtypes.bfloat16
    f = lambda name: np.asarray(inputs[name], np.float32)
    hs = np.ascontiguousarray(f("hidden_states"))
    wT = np.ascontiguousarray(f("in_proj_w").T)                 # [H, 2I]
    winp = np.empty((H, 2 * I), np.float32)
    for g in range(G):
        winp[:, g * 512:g * 512 + 256] = wT[:, g * 256:(g + 1) * 256]
        winp[:, g * 512 + 256:(g + 1) * 512] = wT[:, I + g * 256:I + (g + 1) * 256]
    # block layouts: [128, nblocks*4096]; block g holds 8 lhsT chunk groups
    # [128, 512] = [h_m0, h_m1, gate_m0, gate_m1] so each phase-block is one
    # contiguous DMA
    winb = np.ascontiguousarray(
        winp.reshape(HK, 128, G, 512).transpose(1, 2, 0, 3)
        .reshape(128, G * 4096).astype(bf))
    # layernorm gamma/beta folded into c_fc (exact): silu((hn*g+b) @ W.T + c)
    # = silu(hn @ (W*g).T + (c + W @ b))
    fcw_eff = f("fc_w") * f("ln_g")[None, :]
    fcb_eff = f("fc_b") + f("fc_w") @ f("ln_b")
    sfc_col = np.ascontiguousarray(
        fcw_eff.sum(axis=1, dtype=np.float64).astype(np.float32)
        .reshape(HK, 128).T)
    # fc/out blocks keyed by jp (pair of 128-col output blocks): block jp =
    # [128, 16 kk * 256], chunk (kk, j2) at col kk*256 + j2*128
    fcwb = np.ascontiguousarray(
        fcw_eff.T.reshape(IK, 128, 4, 2, 128).transpose(1, 2, 0, 3, 4)
        .reshape(128, 4 * 4096).astype(bf))
    outwb = np.ascontiguousarray(
        f("out_w").T.reshape(IK, 128, 4, 2, 128).transpose(1, 2, 0, 3, 4)
        .reshape(128, 4 * 4096).astype(bf))
    # cproj blocks keyed by ip: block ip = [128, 8 kk * 256]
    cpwb = np.ascontiguousarray(
        f("cproj_w").T.reshape(HK, 128, 8, 2, 128).transpose(1, 2, 0, 3, 4)
        .reshape(128, 8 * 2048).astype(bf))
    v = f("conv_w").reshape(G, 256, 2, 128, CK)                 # [g, j, cc, i, k]
    cwp = np.ascontiguousarray(
        v.transpose(3, 0, 2, 4, 1).reshape(128, G * 2048).astype(bf))
    shared = dict(
        win=winb, cw=cwp, fcw=fcwb, cpw=cpwb, outw=outwb,
        cbcol=np.ascontiguousarray(f("conv_b").reshape(IK, 128).T),
        cpbcol=np.ascontiguousarray(f("cproj_b").reshape(IK, 128).T),
        fcbcol=np.ascontiguousarray(fcb_eff.reshape(HK, 128).T),
        sfccol=sfc_col,
        onesf=np.ones((128, 128), np.float32),
    )
    ipw_h = f("in_proj_w")[:I]                                  # [I, H]
    in_maps = []
    for c in range(NCORES):
        b, q = divmod(c, QC)
        own = hs[b, q * T:(q + 1) * T]                          # [T, H]
        prev = (np.zeros((3, H), np.float32) if q == 0
                else hs[b, q * T - 3:q * T])
        xTc = np.ascontiguousarray(
            np.concatenate([prev, own], 0).T.astype(bf))        # [H, T+3]
        hh = np.zeros((IK, 128, 4), np.float32)
        hh[:, :, 0:3] = (ipw_h @ prev.T).reshape(IK, 128, 3)    # halo h columns
        hh = np.ascontiguousarray(
            hh.transpose(1, 0, 2).reshape(128, IK * 4).astype(bf))
        in_maps.append(dict(xT=xTc, haloh=hh, **shared))
    return in_maps


def _run(inputs, trace=False):
    from concourse.bass_utils import run_bass_kernel_spmd

    nc = _CACHE.get("nc")
    if nc is None:
        nc = _build()
        _CACHE["nc"] = nc
    in_maps = _pack(inputs)
    try:
        res = run_bass_kernel_spmd(nc, in_maps, core_ids=list(range(NCORES)),
                                   trace=trace)
    except Exception:
        # transient NRT_EXEC_UNIT_UNRECOVERABLE has been observed once after a
        # wedged prior run; one retry has always succeeded
        res = run_bass_kernel_spmd(nc, in_maps, core_ids=list(range(NCORES)),
                                   trace=trace)
    y = np.empty((B, L, H), np.float32)
    for c in range(NCORES):
        b, q = divmod(c, QC)
        y[b, q * T:(q + 1) * T, :] = res.results[c]["yT"].T
    return y, res


def kernel(**inputs) -> np.ndarray:
    y, _ = _run(inputs, trace=False)
    return y


# revision 30
# speedup vs baseline: 1.0281x; 1.0038x over previous
"""Trainium2 Bass kernel for nn_MetaMixer_6717328851330.

Computation (see reference):
    p = x @ W_in.T ; h, gate = split(p) ; gate = silu(gate)
    h = causal_grouped_conv1d(h) + b_conv ; h = h * gate       (residual)
    hn = layernorm_I(h) ; m = silu(hn @ W_fc.T + b_fc) @ W_cp.T + b_cp
    y = (m + residual) @ W_out.T

Strategy: pure data-parallel over the 8192 tokens (B*L), 1024 tokens per
core, no collectives. The causal-conv left halo (3 tokens) is shipped
pre-computed from the host (haloh); the intra-core segment halo comes from
the previous segment's h tiles resident in SBUF.

All matmuls run in bf16 (tolerance is 2e-2; measured bf16 error ~5e-3),
which halves HBM weight traffic AND halves the PE's per-matmul stationary
weight-load time vs fp32r (2-byte vs 4-byte loads). Each weight block is
DMA'd once and consumed by both 512-token segments back-to-back. Bias adds
and LN statistics run on the scalar/vector engines instead of K=1 matmuls,
which on HW cost a full 512-row pass each on the PE (the bottleneck
engine: it runs at ~95% occupancy).

On-core layout: activations live as [channel, token] tiles so every matmul
contracts along SBUF partitions with no transposes (host pre-transposes x
and all weights). PSUM fp32 accumulation throughout; LN stats in fp32.
"""
import sys

sys.path.insert(0, "/opt/trn_rl_repo")
import ml_dtypes
import numpy as np

NCORES = 8
B, L, H, I, G, CK = 2, 4096, 1024, 2048, 8, 4
T = (B * L) // NCORES          # tokens per core
S = 512                        # token segment (= psum bank free dim)
NSEG = T // S
HK = H // 128                  # 8  k-chunks over hidden
IK = I // 128                  # 16 k-chunks over intermediate
QC = NCORES // B               # seq chunks per batch
EPS = 1e-5

_CACHE = {}


def _build():
    import concourse.bacc as bacc
    import concourse.mybir as mybir
    import concourse.tile as tile
    from concourse.alu_op_type import AluOpType

    f32 = mybir.dt.float32
    f32r = mybir.dt.float32r
    bf16 = mybir.dt.bfloat16
    AF = mybir.ActivationFunctionType
    MUL, ADD, SUB = AluOpType.mult, AluOpType.add, AluOpType.subtract

    nc = bacc.Bacc(None, target_bir_lowering=False)

    xT = nc.dram_tensor("xT", [H, T + 3], bf16, kind="ExternalInput")
    win = nc.dram_tensor("win", [128, G * 4096], bf16, kind="ExternalInput")
    cw = nc.dram_tensor("cw", [128, G * 2048], bf16, kind="ExternalInput")
    fcw = nc.dram_tensor("fcw", [128, 4 * 4096], bf16, kind="ExternalInput")
    cpw = nc.dram_tensor("cpw", [128, 8 * 2048], bf16, kind="ExternalInput")
    outw = nc.dram_tensor("outw", [128, 4 * 4096], bf16, kind="ExternalInput")
    haloh = nc.dram_tensor("haloh", [128, IK * 4], bf16, kind="ExternalInput")
    cbcol = nc.dram_tensor("cbcol", [128, IK], f32, kind="ExternalInput")
    cpbcol = nc.dram_tensor("cpbcol", [128, IK], f32, kind="ExternalInput")
    fcbcol = nc.dram_tensor("fcbcol", [128, HK], f32, kind="ExternalInput")
    sfccol = nc.dram_tensor("sfccol", [128, HK], f32, kind="ExternalInput")
    onesf = nc.dram_tensor("onesf", [128, 128], f32r, kind="ExternalInput")
    yT = nc.dram_tensor("yT", [H, T], f32, kind="ExternalOutput")

    with nc.allow_low_precision(reason="bf16 matmul pipeline"), \
         tile.TileContext(nc) as tc, \
         tc.tile_pool(name="sb", bufs=1) as sb, \
         tc.tile_pool(name="ps", bufs=1, space="PSUM") as ps:

        def mm_ps():
            return ps.tile([128, S], f32, tag="mm", bufs=6, name="mmps")

        # ---- inputs + constants. Segment-0 x slices first: the first
        # in_proj chain only needs those 1MB, not the full 2.1MB of x.
        ones128 = sb.tile([1, 128], f32r, tag="ones128", name="ones128")
        nc.sync.dma_start(ones128[:], onesf[0:1, :])
        # x split per segment and over two DMA rings (each dma_start costs
        # ~600ns of its sequencer), segment-0 tiles first: phase A runs
        # segment-major, so the first ~10us of PE work needs only seg-0 x
        xk = [[None] * HK for _ in range(NSEG)]
        for s in range(NSEG):
            for k in range(HK):
                t = sb.tile([128, S + 3], bf16, tag=f"x{s}_{k}",
                            name=f"x{s}_{k}")
                eng = nc.scalar if k % 2 == 0 else nc.gpsimd
                eng.dma_start(t[:], xT[k * 128:(k + 1) * 128,
                                       s * S:s * S + S + 3])
                xk[s][k] = t
        carryall = sb.tile([128, IK * 4], bf16, tag="carryall", name="carryall")
        nc.gpsimd.dma_start(carryall[:], haloh[:])
        ones_col = sb.tile([128, 1], f32r, tag="ones_col", name="ones_col")
        nc.gpsimd.dma_start(ones_col[:], onesf[:, 0:1])
        # p-state warm-up: ~dozen throwaway matmuls on the ones tile so the
        # PE clock is ramped when the first real chain's operands land
        warm = ps.tile([128, S], f32, tag="pba", bufs=2, name="warmps")
        for w in range(6):
            nc.tensor.matmul(warm[:, 0:128], ones128[:], ones128[:],
                             start=(w == 0), stop=(w == 5))
        cb_t = sb.tile([128, IK], f32, tag="cbt", name="cbt")
        nc.gpsimd.dma_start(cb_t[:], cbcol[:])
        cpb_t = sb.tile([128, IK], f32, tag="cpbt", name="cpbt")
        nc.gpsimd.dma_start(cpb_t[:], cpbcol[:])
        fcb_t = sb.tile([128, HK], f32, tag="fcbt", name="fcbt")
        nc.gpsimd.dma_start(fcb_t[:], fcbcol[:])
        sfc_t = sb.tile([128, HK], f32, tag="sfct", name="sfct")
        nc.gpsimd.dma_start(sfc_t[:], sfccol[:])

        res = [[None] * NSEG for _ in range(IK)]
        sumt = [None] * NSEG
        sqsum = [None] * NSEG

        # ---- Phase A: in_proj + causal grouped conv + silu-gate, per group
        for g in range(G):
            wg = sb.tile([128, 4096], bf16, tag="wbig", bufs=3, name="wbig")
            if g == 0:
                # fast start: 8 chunk-sized DMAs so the first matmul only
                # waits on 128KB, not the full 1MB block
                for k in range(HK):
                    nc.sync.dma_start(wg[:, k * 512:(k + 1) * 512],
                                      win[:, k * 512:(k + 1) * 512])
            else:
                # half-block DMAs: the chain's first matmuls start when the
                # first 512KB lands
                nc.sync.dma_start(wg[:, 0:2048], win[:, g * 4096:g * 4096 + 2048])
                nc.sync.dma_start(wg[:, 2048:4096],
                                  win[:, g * 4096 + 2048:(g + 1) * 4096])
            cwt = sb.tile([128, 2048], bf16, tag="cw2", bufs=2, name="cwt")
            nc.scalar.dma_start(cwt[:], cw[:, g * 2048:(g + 1) * 2048])

            # segment-major chain order: all of seg-0's h/gate/conv chains
            # run before seg-1's x tiles are needed
            hts = [[None] * NSEG, [None] * NSEG]
            gss = [[None] * NSEG, [None] * NSEG]
            for s in range(NSEG):
                for m in range(2):
                    i2 = 2 * g + m
                    pm = mm_ps()
                    for k in range(HK):
                        nc.tensor.matmul(pm[:],
                                         wg[:, k * 512 + m * 128:k * 512 + m * 128 + 128],
                                         xk[s][k][:, 3:3 + S],
                                         start=(k == 0), stop=(k == HK - 1))
                    ht = sb.tile([128, S + 3], bf16, tag="hT", bufs=5, name="hT")
                    nc.vector.tensor_copy(ht[:, 3:S + 3], pm[:])
                    if s == 0:
                        nc.vector.tensor_copy(ht[:, 0:3],
                                              carryall[:, i2 * 4:i2 * 4 + 3])
                    else:
                        nc.vector.tensor_copy(ht[:, 0:3],
                                              hts[m][s - 1][:, S:S + 3])
                    hts[m][s] = ht
                for m in range(2):
                    pg = mm_ps()
                    for k in range(HK):
                        nc.tensor.matmul(pg[:],
                                         wg[:, k * 512 + 256 + m * 128:k * 512 + 384 + m * 128],
                                         xk[s][k][:, 3:3 + S],
                                         start=(k == 0), stop=(k == HK - 1))
                    gs = sb.tile([128, S], bf16, tag=f"gsc{s}", bufs=2,
                                 name="gsc")
                    nc.scalar.activation(gs[:], pg[:], AF.Silu)
                    gss[m][s] = gs
                for m in range(2):
                    i2 = 2 * g + m
                    pc = mm_ps()
                    first = True
                    for cc in range(2):
                        for k in range(CK):
                            c0 = cc * 1024 + k * 256 + m * 128
                            nc.tensor.matmul(pc[:], cwt[:, c0:c0 + 128],
                                             hts[cc][s][:, k:k + S],
                                             start=first,
                                             stop=(cc == 1 and k == CK - 1))
                            first = False
                    tb = sb.tile([128, S], bf16, tag="tb", bufs=3, name="tb")
                    nc.scalar.activation(tb[:], pc[:], AF.Identity,
                                         bias=cb_t[:, i2:i2 + 1])
                    rs = sb.tile([128, S], bf16, tag=f"res{i2}_{s}",
                                 name=f"res{i2}_{s}")
                    nc.vector.tensor_tensor(rs[:], tb[:], gss[m][s][:], op=MUL)
                    res[i2][s] = rs

            # ---- LN statistics, part 1: running channel sums / sum-squares
            # interleaved with phase A (cross-partition reduction finishes
            # with a single ones-matmul each below; doing all 16 k-chunks as
            # ones-matmuls would cost 60 more PE passes on the bottleneck)
            for s in range(NSEG):
                r0, r1 = res[2 * g][s], res[2 * g + 1][s]
                q0 = sb.tile([128, S], f32r, tag="sq", bufs=3, name="sq")
                nc.scalar.activation(q0[:], r0[:], AF.Square)
                q1 = sb.tile([128, S], f32r, tag="sq", bufs=3, name="sq")
                nc.scalar.activation(q1[:], r1[:], AF.Square)
                if g == 0:
                    a = sb.tile([128, S], f32r, tag=f"accs{s}", bufs=2,
                                name="accs")
                    nc.vector.tensor_tensor(a[:], r0[:], r1[:], op=ADD)
                    sumt[s] = a
                    aq = sb.tile([128, S], f32r, tag=f"accq{s}", bufs=2,
                                 name="accq")
                    nc.vector.tensor_tensor(aq[:], q0[:], q1[:], op=ADD)
                    sqsum[s] = aq
                else:
                    a1 = sb.tile([128, S], f32r, tag=f"accs{s}", bufs=2,
                                 name="accs")
                    nc.vector.tensor_tensor(a1[:], sumt[s][:], r0[:], op=ADD)
                    a2 = sb.tile([128, S], f32r, tag=f"accs{s}", bufs=2,
                                 name="accs")
                    nc.vector.tensor_tensor(a2[:], a1[:], r1[:], op=ADD)
                    sumt[s] = a2
                    aq1 = sb.tile([128, S], f32r, tag=f"accq{s}", bufs=2,
                                  name="accq")
                    nc.vector.tensor_tensor(aq1[:], sqsum[s][:], q0[:], op=ADD)
                    aq2 = sb.tile([128, S], f32r, tag=f"accq{s}", bufs=2,
                                  name="accq")
                    nc.vector.tensor_tensor(aq2[:], aq1[:], q1[:], op=ADD)
                    sqsum[s] = aq2

        # ---- MLP c_fc, chains only for the first H-block: keeps the PE busy
        # while the LN row math (below) runs on vector/scalar
        def fc_chain(wb, j2, s):
            pm = mm_ps()
            for kk in range(IK):
                nc.tensor.matmul(pm[:],
                                 wb[:, kk * 256 + j2 * 128:kk * 256 + j2 * 128 + 128],
                                 res[kk][s][:],
                                 start=(kk == 0), stop=(kk == IK - 1))
            return pm

        def wblock(src, jp):
            wb = sb.tile([128, 4096], bf16, tag="wbig", bufs=3, name="wbig2")
            nc.sync.dma_start(wb[:, 0:2048], src[:, jp * 4096:jp * 4096 + 2048])
            nc.sync.dma_start(wb[:, 2048:4096],
                              src[:, jp * 4096 + 2048:(jp + 1) * 4096])
            return wb

        fcwb0 = wblock(fcw, 0)
        jp0_psums = {}
        jp0_psums[(0, 0)] = fc_chain(fcwb0, 0, 0)

        # ---- LN statistics, part 2: finish stats. The PE reaches the stat
        # matmuls ~8us into the jp0 chains above, by which time the running
        # accumulators have drained; the j2=1 chains below then cover the
        # serial row-math latency (incl. the Sqrt act-table load) before the
        # pba broadcasts need arow/brow.
        bcA = [None] * NSEG
        bcB = [None] * NSEG
        stat_ps = {}

        def emit_stats(s):
            pst0 = ps.tile([1, S], f32, tag="pba", bufs=2, name="statps")
            nc.tensor.matmul(pst0[:], ones_col[:], sumt[s][:],
                             start=True, stop=True)
            pst1 = ps.tile([1, S], f32, tag="pba", bufs=2, name="statps")
            nc.tensor.matmul(pst1[:], ones_col[:], sqsum[s][:],
                             start=True, stop=True)
            stat_ps[s] = (pst0, pst1)

        emit_stats(0)
        jp0_psums[(0, 1)] = fc_chain(fcwb0, 0, 1)
        emit_stats(1)
        # row math level-by-level across both segments: the two Sqrts stay
        # adjacent on the scalar engine (one act-table load, not two) and the
        # reciprocal uses the fast DVE approximation (exact reciprocal() is
        # 3.3us per row and sat on the critical path)
        rows = {}
        mnegs, msqs, vrows, sds = {}, {}, {}, {}
        for s in range(NSEG):
            mneg = sb.tile([1, S], f32r, tag="mneg", bufs=2, name="mneg")
            nc.vector.tensor_scalar(mneg[:], stat_ps[s][0][:], -1.0 / I,
                                    None, op0=MUL)
            mnegs[s] = mneg
        for s in range(NSEG):
            msq = sb.tile([1, S], f32, tag="msq", bufs=2, name="msq")
            nc.scalar.activation(msq[:], stat_ps[s][0][:], AF.Square)
            msqs[s] = msq
        for s in range(NSEG):
            msq = msqs[s]
            nc.vector.tensor_scalar(msq[:], msq[:], 1.0 / I, None, op0=MUL)
            vrow = sb.tile([1, S], f32, tag="vrow", bufs=2, name="vrow")
            nc.vector.tensor_tensor(vrow[:], stat_ps[s][1][:], msq[:], op=SUB)
            nc.vector.tensor_scalar(vrow[:], vrow[:], 1.0 / I, EPS,
                                    op0=MUL, op1=ADD)
            vrows[s] = vrow
        for s in range(NSEG):
            sd = sb.tile([1, S], f32, tag="sd", bufs=2, name="sd")
            nc.scalar.activation(sd[:], vrows[s][:], AF.Sqrt)
            sds[s] = sd
        for s in range(NSEG):
            ar32 = sb.tile([1, S], f32, tag="ar32", bufs=2, name="ar32")
            nc.vector.reciprocal_approx_fast(ar32[:], sds[s][:])    # rstd
            arow = sb.tile([1, S], f32r, tag="arow", bufs=2, name="arow")
            nc.vector.tensor_copy(arow[:], ar32[:])
            brow = sb.tile([1, S], f32r, tag="brow", bufs=2, name="brow")
            nc.vector.tensor_tensor(brow[:], mnegs[s][:], arow[:], op=MUL)
            rows[s] = (arow, brow)
        for s in range(NSEG):
            jp0_psums[(1, s)] = fc_chain(fcwb0, 1, s)
        for s in range(NSEG):
            arow, brow = rows[s]
            pa = ps.tile([128, S], f32, tag="pba", bufs=2, name="pbaps")
            nc.tensor.matmul(pa[:], ones128[:], arow[:], start=True, stop=True)
            bA = sb.tile([128, S], bf16, tag=f"bcA{s}", name=f"bcA{s}")
            nc.vector.tensor_copy(bA[:], pa[:])
            bcA[s] = bA
            pb = ps.tile([128, S], f32, tag="pba", bufs=2, name="pbaps")
            nc.tensor.matmul(pb[:], ones128[:], brow[:], start=True, stop=True)
            bB = sb.tile([128, S], bf16, tag=f"bcB{s}", name=f"bcB{s}")
            nc.vector.tensor_copy(bB[:], pb[:])
            bcB[s] = bB

        # ---- MLP c_fc: hn = (psum - mean*S_fc_row)*rstd, done per tile as
        # psum*bcA + S_fc[j]*bcB, then silu(+folded bias)
        m1 = [[None] * NSEG for _ in range(HK)]

        def fc_finish(pm, j, s):
            t1 = sb.tile([128, S], bf16, tag="fct1", bufs=2, name="fct1")
            nc.vector.tensor_scalar(t1[:], bcB[s][:], sfc_t[:, j:j + 1], None,
                                    op0=MUL)
            t2 = sb.tile([128, S], bf16, tag="fct2", bufs=2, name="fct2")
            nc.vector.tensor_tensor(t2[:], pm[:], bcA[s][:], op=MUL)
            t3 = sb.tile([128, S], bf16, tag="fct3", bufs=2, name="fct3")
            nc.vector.tensor_tensor(t3[:], t2[:], t1[:], op=ADD)
            o = sb.tile([128, S], bf16, tag=f"m1_{j}_{s}", name=f"m1_{j}_{s}")
            nc.scalar.activation(o[:], t3[:], AF.Silu, bias=fcb_t[:, j:j + 1])
            m1[j][s] = o

        for j2 in range(2):
            for s in range(NSEG):
                fc_finish(jp0_psums[(j2, s)], j2, s)
        for jp in range(1, 4):
            wb = wblock(fcw, jp)
            for j2 in range(2):
                for s in range(NSEG):
                    pm = fc_chain(wb, j2, s)
                    fc_finish(pm, jp * 2 + j2, s)

        # ---- MLP c_proj (H -> I) + bias + residual add
        oin = [[None] * NSEG for _ in range(IK)]
        for ip in range(8):
            wb = sb.tile([128, 2048], bf16, tag="cw2", bufs=2, name="cpwb")
            nc.scalar.dma_start(wb[:], cpw[:, ip * 2048:(ip + 1) * 2048])
            for i2 in range(2):
                i = ip * 2 + i2
                for s in range(NSEG):
                    pm = mm_ps()
                    for kk in range(HK):
                        nc.tensor.matmul(pm[:],
                                         wb[:, kk * 256 + i2 * 128:kk * 256 + i2 * 128 + 128],
                                         m1[kk][s][:],
                                         start=(kk == 0), stop=(kk == HK - 1))
                    tb2 = sb.tile([128, S], bf16, tag="tb", bufs=3, name="tb2")
                    nc.scalar.activation(tb2[:], pm[:], AF.Identity,
                                         bias=cpb_t[:, i:i + 1])
                    oi = sb.tile([128, S], bf16, tag=f"oin{i}_{s}",
                                 name=f"oin{i}_{s}")
                    nc.vector.tensor_tensor(oi[:], tb2[:], res[i][s][:], op=ADD)
                    oin[i][s] = oi

        # ---- out_proj (I -> H)
        for jp in range(4):
            wb = wblock(outw, jp)
            for j2 in range(2):
                j = jp * 2 + j2
                for s in range(NSEG):
                    pm = mm_ps()
                    for kk in range(IK):
                        nc.tensor.matmul(pm[:],
                                         wb[:, kk * 256 + j2 * 128:kk * 256 + j2 * 128 + 128],
                                         oin[kk][s][:],
                                         start=(kk == 0), stop=(kk == IK - 1))
                    yo = sb.tile([128, S], f32, tag="yo", bufs=3, name="yo")
                    nc.vector.tensor_copy(yo[:], pm[:])
                    nc.scalar.dma_start(yT[j * 128:(j + 1) * 128, s * S:(s + 1) * S],
                                        yo[:])

    nc.compile()
    return nc


def _pack(inputs):
    bf = ml_dtypes.bfloat16
    f = lambda name: np.asarray(inputs[name], np.float32)
    hs = np.ascontiguousarray(f("hidden_states"))
    wT = np.ascontiguousarray(f("in_proj_w").T)                 # [H, 2I]
    winp = np.empty((H, 2 * I), np.float32)
    for g in range(G):
        winp[:, g * 512:g * 512 + 256] = wT[:, g * 256:(g + 1) * 256]
        winp[:, g * 512 + 256:(g + 1) * 512] = wT[:, I + g * 256:I + (g + 1) * 256]
    # block layouts: [128, nblocks*4096]; block g holds 8 lhsT chunk groups
    # [128, 512] = [h_m0, h_m1, gate_m0, gate_m1] so each phase-block is one
    # contiguous DMA
    winb = np.ascontiguousarray(
        winp.reshape(HK, 128, G, 512).transpose(1, 2, 0, 3)
        .reshape(128, G * 4096).astype(bf))
    # layernorm gamma/beta folded into c_fc (exact): silu((hn*g+b) @ W.T + c)
    # = silu(hn @ (W*g).T + (c + W @ b))
    fcw_eff = f("fc_w") * f("ln_g")[None, :]
    fcb_eff = f("fc_b") + f("fc_w") @ f("ln_b")
    sfc_col = np.ascontiguousarray(
        fcw_eff.sum(axis=1, dtype=np.float64).astype(np.float32)
        .reshape(HK, 128).T)
    # fc/out blocks keyed by jp (pair of 128-col output blocks): block jp =
    # [128, 16 kk * 256], chunk (kk, j2) at col kk*256 + j2*128
    fcwb = np.ascontiguousarray(
        fcw_eff.T.reshape(IK, 128, 4, 2, 128).transpose(1, 2, 0, 3, 4)
        .reshape(128, 4 * 4096).astype(bf))
    outwb = np.ascontiguousarray(
        f("out_w").T.reshape(IK, 128, 4, 2, 128).transpose(1, 2, 0, 3, 4)
        .reshape(128, 4 * 4096).astype(bf))
    # cproj blocks keyed by ip: block ip = [128, 8 kk * 256]
    cpwb = np.ascontiguousarray(
        f("cproj_w").T.reshape(HK, 128, 8, 2, 128).transpose(1, 2, 0, 3, 4)
        .reshape(128, 8 * 2048).astype(bf))
    v = f("conv_w").reshape(G, 256, 2, 128, CK)                 # [g, j, cc, i, k]
    cwp = np.ascontiguousarray(
        v.transpose(3, 0, 2, 4, 1).reshape(128, G * 2048).astype(bf))
    shared = dict(
        win=winb, cw=cwp, fcw=fcwb, cpw=cpwb, outw=outwb,
        cbcol=np.ascontiguousarray(f("conv_b").reshape(IK, 128).T),
        cpbcol=np.ascontiguousarray(f("cproj_b").reshape(IK, 128).T),
        fcbcol=np.ascontiguousarray(fcb_eff.reshape(HK, 128).T),
        sfccol=sfc_col,
        onesf=np.ones((128, 128), np.float32),
    )
    ipw_h = f("in_proj_w")[:I]                                  # [I, H]
    in_maps = []
    for c in range(NCORES):
        b, q = divmod(c, QC)
        own = hs[b, q * T:(q + 1) * T]                          # [T, H]
        prev = (np.zeros((3, H), np.float32) if q == 0
                else hs[b, q * T - 3:q * T])
        xTc = np.ascontiguousarray(
            np.concatenate([prev, own], 0).T.astype(bf))        # [H, T+3]
        hh = np.zeros((IK, 128, 4), np.float32)
        hh[:, :, 0:3] = (ipw_h @ prev.T).reshape(IK, 128, 3)    # halo h columns
        hh = np.ascontiguousarray(
            hh.transpose(1, 0, 2).reshape(128, IK * 4).astype(bf))
        in_maps.append(dict(xT=xTc, haloh=hh, **shared))
    return in_maps


def _run(inputs, trace=False):
    from concourse.bass_utils import run_bass_kernel_spmd

    nc = _CACHE.get("nc")
    if nc is None:
        nc = _build()
        _CACHE["nc"] = nc
    in_maps = _pack(inputs)
    try:
        res = run_bass_kernel_spmd(nc, in_maps, core_ids=list(range(NCORES)),
                                   trace=trace)
    except Exception:
        # transient NRT_EXEC_UNIT_UNRECOVERABLE has been observed once after a
        # wedged prior run; one retry has always succeeded
        res = run_bass_kernel_spmd(nc, in_maps, core_ids=list(range(NCORES)),
                                   trace=trace)
    y = np.empty((B, L, H), np.float32)
    for c in range(NCORES):
        b, q = divmod(c, QC)
        y[b, q * T:(q + 1) * T, :] = res.results[c]["yT"].T
    return y, res


def kernel(**inputs) -> np.ndarray:
    y, _ = _run(inputs, trace=False)
    return y
